# revision 1
# baseline (speedup 1.0000x reference)
"""DeltaNet Bass kernel for Trainium2, 8-core SPMD.

Sharding: core = (b, h) for b in 0..1, h in 0..3  (b*4 + h).
Each core computes the full per-(batch,head) pipeline and its partial
output projection out_partial[L, D]; the host sums the 4 head-partials
per batch (pure gather/reduce, no compute beyond the sum).

Device pipeline per core (all f32):
  phase1 (per 512-col tile): q/k/v/small projections (PE, lhsT=W, rhs=xT),
          causal conv via diagonal-matmul PSUM accumulation, SiLU evict.
  per 128-token chunk: PE transposes to token-major, l2norm, beta scaling,
          UT-transform T^T via Neumann product (I-M)(I+M^2)(I+M^4)(I+M^8)(I+M^16),
          chunk-local prepass (G, attn^T, w^T, u0), then the serial scan
          (u = u0 - w S, o = q S + attn u, S += k^T u), software-pipelined
          so chunk c+1's independent work fills PE gaps of chunk c's chain.
  phase4/5 (per 512-col tile): FIR short/long via diagonal matmuls,
          transposes, hierarchical gating, RMS norm, output projection.
"""
import numpy as np
import ml_dtypes
from contextlib import ExitStack

import concourse.bass as bass
import concourse.tile as tile
from concourse import bacc, mybir
from concourse.bass_utils import run_bass_kernel_spmd

F32 = mybir.dt.float32
BF16 = mybir.dt.bfloat16
AF = mybir.ActivationFunctionType
ALU = mybir.AluOpType

B, D, H, DK, DV = 2, 1024, 4, 256, 256
CONV_K, FIR_S, FIR_L = 4, 3, 31
CH = 128          # scan chunk (token tile)
NTILE = 512       # column tile for projections / FIR
P = 128
KT = D // P       # 8 contraction tiles over D
GUARD = CONV_K - 1
EPS_RMS = 1e-5
FGUARD = FIR_L    # 31-col guard for FIR windows (>= FIR_L-1, rounded up)
SIM_COMPAT = False  # CoreSim lacks Silu; emulate via sigmoid*x when True


def build(L=4096):
    NT = L // NTILE
    NCH = L // CH
    CPN = NTILE // CH  # chunks per n-tile (4)

    nc = bacc.Bacc("TRN2", target_bir_lowering=False, debug=False, num_devices=8)

    xT_d = nc.dram_tensor("xT", [D, L], F32, kind="ExternalInput").ap()
    wq_d = nc.dram_tensor("wq", [D, DK], F32, kind="ExternalInput").ap()
    wk_d = nc.dram_tensor("wk", [D, DK], F32, kind="ExternalInput").ap()
    wv_d = nc.dram_tensor("wv", [D, DV], F32, kind="ExternalInput").ap()
    wsm_d = nc.dram_tensor("wsm", [D, 5], F32, kind="ExternalInput").ap()
    bias5_d = nc.dram_tensor("bias5", [5], F32, kind="ExternalInput").ap()
    # conv taps per (tensor, pt): [3, 2, 128, 4]
    ctaps_d = nc.dram_tensor("ctaps", [3, 2, P, CONV_K], F32, kind="ExternalInput").ap()
    # fir long-residual diagonal matrices: [pt=2, 31, 128, 128]
    fdiag_d = nc.dram_tensor("fdiag", [2, FIR_L, P, P], BF16, kind="ExternalInput").ap()
    # fir short-residual taps: [2, 128, 3]
    staps_d = nc.dram_tensor("staps", [2, P, FIR_S], F32, kind="ExternalInput").ap()
    wo_d = nc.dram_tensor("wo", [DV, D], F32, kind="ExternalInput").ap()
    ident_d = nc.dram_tensor("ident", [P, P], F32, kind="ExternalInput").ap()
    masklt_d = nc.dram_tensor("masklt", [P, P], F32, kind="ExternalInput").ap()  # strict lower
    maskut_d = nc.dram_tensor("maskut", [P, P], F32, kind="ExternalInput").ap()  # upper incl diag
    out_d = nc.dram_tensor("out", [L, D], F32, kind="ExternalOutput").ap()

    with tile.TileContext(nc) as tc, ExitStack() as ctx:
        # ---------------- pools ----------------
        const = ctx.enter_context(tc.tile_pool(name="const", bufs=1))
        bigw = ctx.enter_context(tc.tile_pool(name="bigw", bufs=1))
        diagp = ctx.enter_context(tc.tile_pool(name="diagp", bufs=1))
        xtp = ctx.enter_context(tc.tile_pool(name="xtp", bufs=1))
        prep = ctx.enter_context(tc.tile_pool(name="prep", bufs=1))
        roll = ctx.enter_context(tc.tile_pool(name="roll", bufs=1))
        colp = ctx.enter_context(tc.tile_pool(name="colp", bufs=1))
        chk = ctx.enter_context(tc.tile_pool(name="chk", bufs=1))
        sp = ctx.enter_context(tc.tile_pool(name="sp", bufs=1))
        gat = ctx.enter_context(tc.tile_pool(name="gat", bufs=1))
        dram = ctx.enter_context(tc.tile_pool(name="dram", bufs=1, space="DRAM"))
        ps_big = ctx.enter_context(tc.tile_pool(name="ps_big", bufs=3, space="PSUM"))
        ps_med = ctx.enter_context(tc.tile_pool(name="ps_med", bufs=5, space="PSUM"))

        # ---------------- DRAM scratch ----------------
        rows_d = dram.tile([5, L], F32, name="rows_sc")
        vTg_d = dram.tile([2, P, FGUARD + L], BF16, name="vTg_sc")
        o_d = dram.tile([NCH, P, DV], F32, name="o_sc")
        vb_d = dram.tile([NCH, P, DV], F32, name="vb_sc")

        # ---------------- constants ----------------
        # big weights first: phase1's first matmuls need wq + xt(0)
        wq = bigw.tile([P, KT, DK], F32, tag="bw", bufs=3)
        nc.sync.dma_start(wq[:], wq_d.rearrange("(kt p) m -> p kt m", p=P))
        wk = bigw.tile([P, KT, DK], F32, tag="bw", bufs=3)
        nc.sync.dma_start(wk[:], wk_d.rearrange("(kt p) m -> p kt m", p=P))
        wv = bigw.tile([P, KT, DV], F32, tag="bw", bufs=3)
        nc.sync.dma_start(wv[:], wv_d.rearrange("(kt p) m -> p kt m", p=P))
        ident = const.tile([P, P], F32)
        nc.sync.dma_start(ident[:], ident_d)
        masklt = const.tile([P, P], F32)
        nc.sync.dma_start(masklt[:], masklt_d)
        maskut = const.tile([P, P], F32)
        nc.sync.dma_start(maskut[:], maskut_d)
        bias5 = const.tile([5, 1], F32)
        nc.sync.dma_start(bias5[:], bias5_d.rearrange("(m o) -> m o", o=1))
        wsm = const.tile([P, KT, 5], F32)
        nc.sync.dma_start(wsm[:], wsm_d.rearrange("(kt p) m -> p kt m", p=P))
        zeros31 = const.tile([P, FGUARD], F32)
        nc.vector.memset(zeros31[:], 0.0)
        zeros31b = const.tile([P, FGUARD], BF16)
        nc.vector.memset(zeros31b[:], 0.0)
        eps_l2 = const.tile([P, 1], F32)
        nc.vector.memset(eps_l2[:], 1e-6)
        eps_rms = const.tile([P, 1], F32)
        nc.vector.memset(eps_rms[:], EPS_RMS)


        ctaps = const.tile([P, 3, 2, CONV_K], F32, name="ctaps")
        nc.sync.dma_start(ctaps[:], ctaps_d.rearrange("t pt p j -> p t pt j"))

        # zero the vT guard region in DRAM
        for pt in range(2):
            nc.sync.dma_start(vTg_d[pt, :, 0:FGUARD], zeros31b[:])

        # ---------------- persistent state ----------------
        S0 = sp.tile([P, DV], F32)
        S1 = sp.tile([P, DV], F32)
        S_sb = [S0, S1]
        nc.vector.memset(S0[:], 0.0)
        nc.vector.memset(S1[:], 0.0)

        # rolling post-silu chan-major tiles, per tensor per pt
        def roll_tile(tag):
            bufs = 2 if (tag.startswith("rollq") or tag.startswith("rollk")) else 1
            return roll.tile([P, NTILE], F32, tag=tag, bufs=bufs, name=tag)

        def rollkq_tile(pt):
            return roll.tile([P, 2 * NTILE], F32, tag=f"rollkq{pt}", bufs=2, name=f"rollkq{pt}")

        # pre-conv rolling tiles (guarded by GUARD cols)
        def pre_tile(tag):
            return prep.tile([P, GUARD + NTILE], F32, tag=tag, bufs=2, name=tag)

        prev_pre = {}   # tag -> tile (for guard copy)
        cur_roll = {}   # (t, pt) -> tile for current n

        TENS = ("q", "k", "v")
        W_OF = {"q": wq, "k": wk, "v": wv}

        # ---------------- per-n phase 1 ----------------
        def phase1_real(n):
            xts = []
            for kt in range(KT):
                xt = xtp.tile([P, NTILE], F32, tag=f"xt{kt}", bufs=1, name=f"xt{kt}")
                nc.sync.dma_start(xt[:], xT_d[kt * P:(kt + 1) * P, n * NTILE:(n + 1) * NTILE])
                xts.append(xt)
            for t in TENS:
                for pt in range(2):
                    ps = ps_big.tile([P, NTILE], F32, tag="psb", name=f"ps_{t}{pt}")
                    for kt in range(KT):
                        nc.tensor.matmul(ps[:], W_OF[t][:, kt, pt * P:(pt + 1) * P], xts[kt][:],
                                         start=(kt == 0), stop=(kt == KT - 1))
                    key = f"pre{t}{pt}"
                    pre = pre_tile(key)
                    if n == 0:
                        nc.scalar.mul(pre[:, 0:GUARD], zeros31[:, 0:GUARD], 0.0)
                    else:
                        nc.scalar.copy(pre[:, 0:GUARD], prev_pre[key][:, NTILE:NTILE + GUARD])
                    nc.scalar.copy(pre[:, GUARD:], ps[:])
                    prev_pre[key] = pre
            # small projection: [5, NTILE] in one ps_big bank
            ps5 = ps_big.tile([P, NTILE], F32, tag="psb", name="ps5")
            for kt in range(KT):
                nc.tensor.matmul(ps5[:5, :], wsm[:, kt, :], xts[kt][:],
                                 start=(kt == 0), stop=(kt == KT - 1))
            rows_sb = colp.tile([5, NTILE], F32, tag="rows_sb", bufs=1, name="rows_sb")
            nc.scalar.activation(rows_sb[:], ps5[:5, :], AF.Identity, bias=bias5[:])
            nc.sync.dma_start(rows_d[:, n * NTILE:(n + 1) * NTILE], rows_sb[:])
            # conv (DVE scalar_tensor_tensor chain) + silu -> rolling chan-major tiles
            # k/q land interleaved per chunk into rollkq: [(128 k | 128 q) x 4 chunks]
            cur_roll[("kq", 0)] = rollkq_tile(0)
            cur_roll[("kq", 1)] = rollkq_tile(1)
            for t in TENS:
                for pt in range(2):
                    pre = prev_pre[f"pre{t}{pt}"]
                    ti = TENS.index(t)
                    acc = roll.tile([P, NTILE], F32, tag="cacc", bufs=2, name="cacc")
                    nc.vector.tensor_scalar_mul(acc[:], pre[:, 0:NTILE], ctaps[:, ti, pt, 0:1])
                    for j in range(1, CONV_K):
                        nc.vector.scalar_tensor_tensor(acc[:], pre[:, j:j + NTILE],
                                                       ctaps[:, ti, pt, j:j + 1], acc[:],
                                                       op0=ALU.mult, op1=ALU.add)
                    if t == "v":
                        rt = roll_tile(f"roll{t}{pt}")
                        if SIM_COMPAT:
                            nc.scalar.activation(rt[:], acc[:], AF.Sigmoid)
                            nc.vector.tensor_tensor(rt[:], rt[:], acc[:], ALU.mult)
                        else:
                            nc.scalar.activation(rt[:], acc[:], AF.Silu)
                        cur_roll[(t, pt)] = rt
                    else:
                        rt = cur_roll[("kq", pt)]
                        koff = 0 if t == "k" else CH
                        for ci in range(CPN):
                            dst = rt[:, ci * 2 * CH + koff: ci * 2 * CH + koff + CH]
                            src = acc[:, ci * CH:(ci + 1) * CH]
                            if SIM_COMPAT:
                                nc.scalar.activation(dst, src, AF.Sigmoid)
                                nc.vector.tensor_tensor(dst, dst, src, ALU.mult)
                            else:
                                nc.scalar.activation(dst, src, AF.Silu)
                    cur_roll[(t, pt)] = rt
            # spill vT (guarded) to DRAM in bf16 for the FIR phase
            for pt in range(2):
                vbf = roll.tile([P, NTILE], BF16, tag=f"vbf{pt}", bufs=2, name=f"vbf{pt}")
                nc.vector.tensor_copy(vbf[:], cur_roll[("v", pt)][:])
                nc.sync.dma_start(vTg_d[pt, :, FGUARD + n * NTILE: FGUARD + (n + 1) * NTILE],
                                  vbf[:])

        # ---------------- per-chunk norms ----------------
        def norms(c):
            n_in = c // CPN
            off = (c % CPN) * CH
            # beta column for this n-block (computed once per n)
            if c % CPN == 0:
                beta_n = colp.tile([P, CPN], F32, tag="beta_n", bufs=2, name="beta_n")
                raw = colp.tile([P, CPN], F32, tag="beta_raw", bufs=2, name="beta_raw")
                nc.sync.dma_start(raw[:], rows_d[0, n_in * NTILE:(n_in + 1) * NTILE]
                                  .rearrange("(nn p) -> p nn", p=P))
                nc.scalar.activation(beta_n[:], raw[:], AF.Sigmoid)
                norms.beta_n = beta_n
            beta_c = norms.beta_n[:, c % CPN:c % CPN + 1]

            # v: transpose + beta-scale -> vb_t [128, 256]; DMA to vb_d
            vb_t = chk.tile([P, DV], F32, tag="vb_t", bufs=2, name="vb_t")
            for pt in range(2):
                tp = ps_med.tile([P, DV], F32, tag="psm", name="tp_v")
                nc.tensor.transpose(tp[:, :P], cur_roll[("v", pt)][:, off:off + CH], ident[:])
                nc.vector.tensor_scalar_mul(vb_t[:, pt * P:(pt + 1) * P], tp[:, :P], beta_c)
            nc.sync.dma_start(vb_d[c], vb_t[:])

            res = {"vb": vb_t, "beta": beta_c}
            # raw chan-major chunk slices into the interleaved rollkq tiles
            ci = c % CPN
            res["kTsl"] = [cur_roll[("kq", pt)][:, ci * 2 * CH: ci * 2 * CH + CH] for pt in range(2)]
            res["qTsl"] = [cur_roll[("kq", pt)][:, ci * 2 * CH + CH: (ci + 1) * 2 * CH] for pt in range(2)]
            res["kqTsl"] = [cur_roll[("kq", pt)][:, ci * 2 * CH: (ci + 1) * 2 * CH] for pt in range(2)]
            # token-major raw q/k for the l2 norms (toks first, then both norm chains
            # so the ACT SQRTs are adjacent and the table loads once)
            toks = {}
            ssqs = {}
            for t, sl in (("q", res["qTsl"]), ("k", res["kTsl"])):
                tok = chk.tile([P, DV], F32, tag=f"{t}tok", bufs=2, name=f"{t}tok")
                for pt in range(2):
                    tp = ps_med.tile([P, DV], F32, tag="psm", name=f"tp_{t}")
                    nc.tensor.transpose(tp[:, :P], sl[pt], ident[:])
                    nc.scalar.copy(tok[:, pt * P:(pt + 1) * P], tp[:, :P])
                toks[t] = tok
                scr = chk.tile([P, DV], F32, tag="sq_scr", bufs=2, name="sq_scr")
                ssq = chk.tile([P, 1], F32, tag=f"ssq{t}", bufs=2, name=f"ssq{t}")
                nc.vector.scalar_tensor_tensor(scr[:], tok[:], 1.0, tok[:],
                                               op0=ALU.mult, op1=ALU.mult, accum_out=ssq[:])
                ssqs[t] = ssq
            sroots = {}
            for t in ("q", "k"):
                sroot = chk.tile([P, 1], F32, tag=f"sroot{t}", bufs=2, name=f"sroot{t}")
                nc.scalar.activation(sroot[:], ssqs[t][:], AF.Sqrt, bias=eps_l2[:])
                sroots[t] = sroot
            for t in ("q", "k"):
                rinv = chk.tile([P, 1], F32, tag=f"rinv{t}", bufs=3, name=f"rinv{t}")
                nc.vector.reciprocal(rinv[:], sroots[t][:])
                res["rinv" + t] = rinv
                if t == "k":
                    khat = chk.tile([P, DV], F32, tag="khat", bufs=2, name="khat")
                    nc.vector.tensor_scalar_mul(khat[:], toks[t][:], rinv[:])
                    res["khat"] = khat
            khatb = chk.tile([P, DV], F32, tag="khatb", bufs=2, name="khatb")
            nc.vector.tensor_scalar_mul(khatb[:], res["khat"], beta_c)
            res["khatb"] = khatb
            return res

        # ---------------- per-chunk prepass ----------------
        def mm_small(lhsT, rhs, name, evict="scalar"):
            ps = ps_med.tile([P, DV], F32, tag="psm", name=f"ps_{name}")
            nc.tensor.matmul(ps[:, :P], lhsT, rhs, start=True, stop=True)
            sb = chk.tile([P, P], F32, tag=name, bufs=1, name=name)
            nc.vector.tensor_copy(sb[:], ps[:, :P])
            return sb

        def prepass(c, nr):
            beta_c = nr["beta"]
            rinvk, rinvq = nr["rinvk"], nr["rinvq"]
            kTsl, qTsl = nr["kTsl"], nr["qTsl"]
            # [Graw | Braw] = kraw @ [kraw | qraw]^T in one N=256 stream per pt
            gps = ps_med.tile([P, DV], F32, tag="psm", name="gps")
            for pt in range(2):
                nc.tensor.matmul(gps[:], kTsl[pt], nr["kqTsl"][pt],
                                 start=(pt == 0), stop=(pt == 1))
            bps = gps
            # N1 = tril_strict * rowscale_{beta*rinvk}(Graw); M = N1^T diag(rinvk); N = M^T
            br = chk.tile([P, 1], F32, tag="br", bufs=2, name="br")
            nc.vector.tensor_tensor(br[:], beta_c, rinvk[:], op=ALU.mult)
            N1 = chk.tile([P, P], F32, tag="N1", bufs=2, name="N1")
            nc.vector.scalar_tensor_tensor(N1[:], gps[:, :P], br[:], masklt[:],
                                           op0=ALU.mult, op1=ALU.mult)
            mps = ps_med.tile([P, DV], F32, tag="psm", name="mps")
            nc.tensor.transpose(mps[:, :P], N1[:], ident[:])
            Mt = chk.tile([P, P], F32, tag="Mt", bufs=1, name="Mt")
            nc.vector.tensor_scalar_mul(Mt[:], mps[:, :P], rinvk[:])
            nps = ps_med.tile([P, DV], F32, tag="psm", name="nps")
            nc.tensor.transpose(nps[:, :P], Mt[:], ident[:])
            Nt = chk.tile([P, P], F32, tag="Nt", bufs=1, name="Nt")
            nc.scalar.copy(Nt[:], nps[:, :P])
            # powers
            N2 = mm_small(Mt[:], Nt[:], "N2")
            M2 = mm_small(Nt[:], Mt[:], "M2")
            N4 = mm_small(M2[:], N2[:], "N4")
            M4 = mm_small(N2[:], M2[:], "M4")
            N8 = mm_small(M4[:], N4[:], "N8")
            M8 = mm_small(N4[:], M4[:], "M8")
            N16 = mm_small(M8[:], N8[:], "N16")
            # P chain: P1 = I - M ; P_{j+1} = P_j + M^(2^j) @ P_j (lhsT = N^(2^j))
            P1 = chk.tile([P, P], F32, tag="P1", bufs=1, name="P1")
            nc.vector.tensor_tensor(P1[:], ident[:], Mt[:], op=ALU.subtract)
            Pc = P1
            for Npow, nm in ((N2, "P2"), (N4, "P3"), (N8, "P4"), (N16, "TTt")):
                pps = ps_med.tile([P, DV], F32, tag="psm", name=f"pps_{nm}")
                nc.tensor.matmul(pps[:, :P], Npow[:], Pc[:], start=True, stop=True)
                nxt = chk.tile([P, P], F32, tag=nm, bufs=1 if nm != "TTt" else 2, name=nm)
                nc.vector.tensor_tensor(nxt[:], Pc[:], pps[:, :P], op=ALU.add)
                Pc = nxt
            TTt = Pc
            negTT = chk.tile([P, P], F32, tag="negTT", bufs=2, name="negTT")
            nc.vector.tensor_scalar_mul(negTT[:], TTt[:], -1.0)
            # attn^T = rowscale_{rinvk}(triu_incl * Braw)
            attnT = chk.tile([P, P], F32, tag="attnT", bufs=2, name="attnT")
            nc.vector.scalar_tensor_tensor(attnT[:], bps[:, P:], rinvk[:], maskut[:],
                                           op0=ALU.mult, op1=ALU.mult)
            # w^T(neg): [128, 2, 128]
            wTn = chk.tile([P, 2, CH], F32, tag="wTn", bufs=2, name="wTn")
            for kt in range(2):
                wps = ps_med.tile([P, DV], F32, tag="psm", name="wps")
                nc.tensor.matmul(wps[:, :P], nr["khatb"][:, kt * P:(kt + 1) * P], negTT[:],
                                 start=True, stop=True)
                nc.scalar.copy(wTn[:, kt, :], wps[:, :P])
            # u0 = T @ vb : lhsT = T^T
            ups = ps_med.tile([P, DV], F32, tag="psm", name="ups")
            nc.tensor.matmul(ups[:], TTt[:], nr["vb"][:], start=True, stop=True)
            u0 = chk.tile([P, DV], F32, tag="u0", bufs=2, name="u0")
            nc.scalar.copy(u0[:], ups[:])
            return {"TTt": TTt, "attnT": attnT, "wTn": wTn, "u0": u0,
                    "qTsl": qTsl, "khat": nr["khat"], "rinvq": rinvq}

        # ---------------- per-chunk serial pass ----------------
        def serial(c, pr):
            if c == 0:
                u_sb = pr["u0"]
            else:
                ups = ps_med.tile([P, DV], F32, tag="psm", name="ups_s")
                for kt in range(2):
                    nc.tensor.matmul(ups[:], pr["wTn"][:, kt, :], S_sb[kt][:],
                                     start=(kt == 0), stop=(kt == 1))
                u_sb = chk.tile([P, DV], F32, tag="u_sb", bufs=2, name="u_sb")
                nc.vector.tensor_tensor(u_sb[:], ups[:], pr["u0"][:], op=ALU.add)
            # o
            ops = ps_med.tile([P, DV], F32, tag="psm", name="ops")
            if c == 0:
                nc.tensor.matmul(ops[:], pr["attnT"][:], u_sb[:], start=True, stop=True)
            else:
                for kt in range(2):
                    nc.tensor.matmul(ops[:], pr["qTsl"][kt], S_sb[kt][:],
                                     start=(kt == 0), stop=False)
                nc.tensor.matmul(ops[:], pr["attnT"][:], u_sb[:], start=False, stop=True)
            o_sb = chk.tile([P, DV], F32, tag="o_sb", bufs=2, name="o_sb")
            nc.vector.tensor_scalar_mul(o_sb[:], ops[:], pr["rinvq"][:])
            nc.sync.dma_start(o_d[c], o_sb[:])
            # S += k^T u
            for kt in range(2):
                dps = ps_med.tile([P, DV], F32, tag="psm", name=f"dps{kt}")
                nc.tensor.matmul(dps[:], pr["khat"][:, kt * P:(kt + 1) * P], u_sb[:],
                                 start=True, stop=True)
                if c == 0:
                    nc.scalar.copy(S_sb[kt][:], dps[:])
                else:
                    nc.vector.tensor_tensor(S_sb[kt][:], S_sb[kt][:], dps[:], op=ALU.add)

        # ================= emit phases 1-3 =================
        pending = None  # (c, prepass_result)
        for n in range(NT):
            phase1_real(n)
            if n == NT - 1:
                # gates/FIR weight setup emitted here so the DMAs + column math
                # overlap the scan of the last chunks
    
                wo = bigw.tile([P, 2, D], F32, tag="bw", bufs=3, name="wo")
                nc.sync.dma_start(wo[:], wo_d.rearrange("(kt p) m -> p kt m", p=P))
                fdiag = diagp.tile([P, 2, FIR_L, P], BF16, tag="diag", bufs=1, name="fdiag")
                nc.sync.dma_start(fdiag[:], fdiag_d.rearrange("pt j p q -> p pt j q"))
                staps = const.tile([P, 2, FIR_S], F32, name="staps")
                nc.sync.dma_start(staps[:], staps_d.rearrange("pt p j -> p pt j"))

                def col_from_row(r, name, act=None):
                    t = colp.tile([P, NCH], F32, tag=name, bufs=1, name=name)
                    nc.sync.dma_start(t[:], rows_d[r].rearrange("(nn p) -> p nn", p=P))
                    if act is not None:
                        nc.scalar.activation(t[:], t[:], act)
                    return t

                wg_c = col_from_row(1, "wg_c", AF.Sigmoid)
                l0_c = col_from_row(2, "l0_c")
                l1_c = col_from_row(3, "l1_c")
                l2_c = col_from_row(4, "l2_c")
                bfull_c = col_from_row(0, "bfull_c", AF.Sigmoid)
                mx = colp.tile([P, NCH], F32, tag="mx", bufs=1, name="mx")
                nc.vector.tensor_tensor(mx[:], l0_c[:], l1_c[:], op=ALU.max)
                nc.vector.tensor_tensor(mx[:], mx[:], l2_c[:], op=ALU.max)
                e0 = colp.tile([P, NCH], F32, tag="e0", bufs=1, name="e0")
                e1 = colp.tile([P, NCH], F32, tag="e1", bufs=1, name="e1")
                e2 = colp.tile([P, NCH], F32, tag="e2", bufs=1, name="e2")
                for src, dst in ((l0_c, e0), (l1_c, e1), (l2_c, e2)):
                    nc.vector.tensor_tensor(dst[:], src[:], mx[:], op=ALU.subtract)
                    nc.scalar.activation(dst[:], dst[:], AF.Exp)
                esum = colp.tile([P, NCH], F32, tag="esum", bufs=1, name="esum")
                nc.vector.tensor_tensor(esum[:], e0[:], e1[:], op=ALU.add)
                nc.vector.tensor_tensor(esum[:], esum[:], e2[:], op=ALU.add)
                erec = colp.tile([P, NCH], F32, tag="erec", bufs=1, name="erec")
                nc.vector.reciprocal(erec[:], esum[:])
                p1 = colp.tile([P, NCH], F32, tag="p1", bufs=1, name="p1")
                p2 = colp.tile([P, NCH], F32, tag="p2", bufs=1, name="p2")
                for src, dst in ((e1, p1), (e2, p2)):
                    nc.vector.tensor_tensor(dst[:], src[:], erec[:], op=ALU.mult)
                brec = colp.tile([P, NCH], F32, tag="brec", bufs=1, name="brec")
                nc.vector.reciprocal(brec[:], bfull_c[:])
                wg1m = colp.tile([P, NCH], F32, tag="wg1m", bufs=1, name="wg1m")
                nc.vector.tensor_scalar(wg1m[:], wg_c[:], -1.0, 1.0, op0=ALU.mult, op1=ALU.add)


            for c in range(n * CPN, (n + 1) * CPN):
                nr = norms(c)
                pr = prepass(c, nr)
                if pending is not None:
                    serial(pending[0], pending[1])
                pending = (c, pr)
        serial(pending[0], pending[1])

        # ================= phases 4/5: FIR + gating + output =================
        def emit_outproj(lt, on):
            onT = gat.tile([P, 2, CH], F32, tag="onT", bufs=2, name="onT")
            for pt in range(2):
                tp = ps_med.tile([P, DV], F32, tag="psm", name="tp_on")
                nc.tensor.transpose(tp[:, :P], on[:, pt * P:(pt + 1) * P], ident[:])
                nc.scalar.copy(onT[:, pt, :], tp[:, :P])
            out_sb = gat.tile([P, D], F32, tag="out_sb", bufs=2, name="out_sb")
            for nt2 in range(2):
                ops = ps_big.tile([P, NTILE], F32, tag="psb", name="ops_o")
                for kt in range(2):
                    nc.tensor.matmul(ops[:], onT[:, kt, :], wo[:, kt, nt2 * NTILE:(nt2 + 1) * NTILE],
                                     start=(kt == 0), stop=(kt == 1))
                nc.scalar.copy(out_sb[:, nt2 * NTILE:(nt2 + 1) * NTILE], ops[:])
            nc.sync.dma_start(out_d[lt * CH:(lt + 1) * CH, :], out_sb[:])

        def load_lt(lt):
            o_lt = gat.tile([P, DV], F32, tag="o_lt", bufs=8, name="o_lt")
            nc.sync.dma_start(o_lt[:], o_d[lt])
            vb_lt = gat.tile([P, DV], F32, tag="vb_lt", bufs=8, name="vb_lt")
            nc.sync.dma_start(vb_lt[:], vb_d[lt])
            return o_lt, vb_lt

        def stageA(lt, fch, loads):
            off = (lt % CPN) * CH
            toks = {}
            for f in ("ls", "ll"):
                tokt = gat.tile([P, DV], F32, tag=f"{f}tok", bufs=4, name=f"{f}tok")
                for pt in range(2):
                    tp = ps_med.tile([P, DV], F32, tag="psm", name=f"tp_{f}")
                    nc.tensor.transpose(tp[:, :P], fch[(f, pt)][:, off:off + CH], ident[:])
                    nc.scalar.copy(tokt[:, pt * P:(pt + 1) * P], tp[:, :P])
                toks[f] = tokt
            o_lt, vb_lt = loads[lt]
            return dict(toks=toks, o_lt=o_lt, vb_lt=vb_lt)

        def stageB(lt, a):
            cs = lambda t: t[:, lt:lt + 1]
            toks, o_lt, vb_lt = a["toks"], a["o_lt"], a["vb_lt"]
            mix = gat.tile([P, DV], F32, tag="gtmp", bufs=5, name="mix")
            nc.vector.tensor_scalar_mul(mix[:], vb_lt[:], cs(brec))
            mix2 = gat.tile([P, DV], F32, tag="gtmp", bufs=5, name="mix2")
            nc.vector.scalar_tensor_tensor(mix2[:], toks["ls"][:], cs(p1), mix[:],
                                           op0=ALU.mult, op1=ALU.add)
            mix3 = gat.tile([P, DV], F32, tag="gtmp", bufs=5, name="mix3")
            nc.vector.scalar_tensor_tensor(mix3[:], toks["ll"][:], cs(p2), mix2[:],
                                           op0=ALU.mult, op1=ALU.add)
            tmp = gat.tile([P, DV], F32, tag="gtmp", bufs=5, name="tmpg")
            nc.vector.tensor_scalar_mul(tmp[:], mix3[:], cs(wg1m))
            om = gat.tile([P, DV], F32, tag="gtmp", bufs=5, name="om")
            nc.vector.scalar_tensor_tensor(om[:], o_lt[:], cs(wg_c), tmp[:],
                                           op0=ALU.mult, op1=ALU.add)
            scr = gat.tile([P, DV], F32, tag="scr_g", bufs=2, name="scr_g")
            ssq = gat.tile([P, 1], F32, tag="ssq_g", bufs=2, name="ssq_g")
            nc.vector.scalar_tensor_tensor(scr[:], om[:], 1.0, om[:],
                                           op0=ALU.mult, op1=ALU.mult, accum_out=ssq[:])
            srt = gat.tile([P, 1], F32, tag="srt_g", bufs=2, name="srt_g")
            nc.scalar.activation(srt[:], ssq[:], AF.Sqrt, bias=eps_rms[:], scale=1.0 / DV)
            rin = gat.tile([P, 1], F32, tag="rin_g", bufs=2, name="rin_g")
            nc.vector.reciprocal(rin[:], srt[:])
            on = gat.tile([P, DV], F32, tag="on_g", bufs=4, name="on_g")
            nc.vector.tensor_scalar_mul(on[:], om[:], rin[:])
            return on

        pend_block = None  # (lts, fch, loads) of the previous n-block
        for n in range(NT):
            fch = {}
            As = None
            if pend_block is not None:
                p_lts, p_fch, p_loads = pend_block
                As = [(lt, stageA(lt, p_fch, p_loads)) for lt in p_lts]
            for pt in range(2):
                vwin = gat.tile([P, FGUARD + NTILE], BF16, tag="vwin", bufs=3, name="vwin")
                nc.sync.dma_start(vwin[:], vTg_d[pt, :, n * NTILE: FGUARD + (n + 1) * NTILE])
                # long-residual FIR on PE (bf16 diag matmuls)
                ps = ps_big.tile([P, NTILE], F32, tag="psb", name="ps_ll")
                base = FGUARD - FIR_L + 1
                for j in range(FIR_L):
                    nc.tensor.matmul(ps[:], fdiag[:, pt, j, :], vwin[:, base + j: base + j + NTILE],
                                     start=(j == 0), stop=(j == FIR_L - 1))
                sb = gat.tile([P, NTILE], F32, tag="llch", bufs=3, name="llch")
                nc.vector.tensor_copy(sb[:], ps[:])
                fch[("ll", pt)] = sb
                # short-residual FIR on DVE (3-tap STT chain)
                sbs = gat.tile([P, NTILE], F32, tag="lsch", bufs=3, name="lsch")
                bs = FGUARD - FIR_S + 1
                nc.vector.tensor_scalar_mul(sbs[:], vwin[:, bs:bs + NTILE], staps[:, pt, 0:1])
                for j in range(1, FIR_S):
                    nc.vector.scalar_tensor_tensor(sbs[:], vwin[:, bs + j:bs + j + NTILE],
                                                   staps[:, pt, j:j + 1], sbs[:],
                                                   op0=ALU.mult, op1=ALU.add)
                fch[("ls", pt)] = sbs
            lts_n = list(range(n * CPN, (n + 1) * CPN))
            loads_n = {lt: load_lt(lt) for lt in lts_n}
            if As is not None:
                for lt, a in As:
                    on = stageB(lt, a)
                    emit_outproj(lt, on)
            pend_block = (lts_n, fch, loads_n)
        p_lts, p_fch, p_loads = pend_block
        As = [(lt, stageA(lt, p_fch, p_loads)) for lt in p_lts]
        ons = [(lt, stageB(lt, a)) for lt, a in As]
        for lt, on in ons:
            emit_outproj(lt, on)

    nc.compile()
    return nc




# ---------------- host side ----------------

def _diag_stack(taps):
    """taps [C, K] -> [2, K, 128, 128] diag matrices."""
    C, K = taps.shape
    out = np.zeros((2, K, P, P), np.float32)
    for pt in range(2):
        for j in range(K):
            np.fill_diagonal(out[pt, j], taps[pt * P:(pt + 1) * P, j])
    return out


def make_core_inputs(inputs, b, h, L):
    f = lambda a: np.ascontiguousarray(np.asarray(a, np.float32))
    x = f(inputs['hidden_states'])[b]          # [L, D]
    temp = float(np.exp(np.asarray(inputs['log_temp'], np.float64)[h]))
    wsm = np.concatenate([
        f(inputs['Wb'])[:, h:h + 1],
        f(inputs['Wg'])[:, h:h + 1],
        f(inputs['Wl'])[:, 3 * h:3 * h + 3] / temp], axis=1)
    bias5 = np.array([0.0, float(np.asarray(inputs['bg'], np.float64)[h]),
                      *(np.asarray(inputs['bl'], np.float64)[3 * h:3 * h + 3] / temp)],
                     np.float32)
    ct = np.stack([
        f(inputs['conv_q'])[h * DK:(h + 1) * DK].reshape(2, P, CONV_K),
        f(inputs['conv_k'])[h * DK:(h + 1) * DK].reshape(2, P, CONV_K),
        f(inputs['conv_v'])[h * DV:(h + 1) * DV].reshape(2, P, CONV_K)])  # [3, 2, 128, 4]
    # residual FIR taps: fir = delta + r  ->  local = v + FIR_r(v); softmax sums to 1
    fs = f(inputs['fir_short'])[h].copy()   # [DV, 3]
    fs[:, -1] -= 1.0
    fl = f(inputs['fir_long'])[h].copy()    # [DV, 31]
    fl[:, -1] -= 1.0
    fd = _diag_stack(fl).astype(ml_dtypes.bfloat16)
    st = fs.reshape(2, P, FIR_S)
    wo = f(inputs['rms_w'])[:, None] * f(inputs['Wo'])[h * DV:(h + 1) * DV]
    return dict(
        xT=np.ascontiguousarray(x.T),
        wq=np.ascontiguousarray(f(inputs['Wq'])[:, h * DK:(h + 1) * DK]),
        wk=np.ascontiguousarray(f(inputs['Wk'])[:, h * DK:(h + 1) * DK]),
        wv=np.ascontiguousarray(f(inputs['Wv'])[:, h * DV:(h + 1) * DV]),
        wsm=wsm.astype(np.float32), bias5=bias5,
        ctaps=ct, fdiag=fd, staps=st.astype(np.float32), wo=wo.astype(np.float32),
        ident=np.eye(P, dtype=np.float32),
        masklt=np.tril(np.ones((P, P), np.float32), -1),
        maskut=np.triu(np.ones((P, P), np.float32), 0),
    )


_NC_CACHE = {}


def _get_nc(L):
    if L not in _NC_CACHE:
        _NC_CACHE[L] = build(L)
    return _NC_CACHE[L]


def kernel(**inputs):
    x = np.asarray(inputs['hidden_states'])
    Bx, L, _ = x.shape
    nc = _get_nc(L)
    in_maps = [make_core_inputs(inputs, c // H, c % H, L) for c in range(8)]
    res = run_bass_kernel_spmd(nc, in_maps, core_ids=list(range(8)))
    out = np.zeros((Bx, L, D), np.float32)
    for c in range(8):
        out[c // H] += res.results[c]['out']
    return out



# revision 9
# speedup vs baseline: 1.5116x; 1.5116x over previous
"""DeltaNet Bass kernel for Trainium2, 8-core SPMD — fp16 matmul pipeline.

Sharding: core = (b, h) for b in 0..1, h in 0..3  (b*4 + h).
Each core computes the full per-(batch,head) pipeline and its partial
output projection out_partial[L, D] in fp16; the host sums the 4
head-partials per batch.

Device pipeline per core (PSUM accumulate fp32, operands fp16):
  phase A (per 512-col tile, all 8 tiles first): q/k/v/small projections
          (PE, fp16), causal 4-tap conv (DVE STT chain, fp16 4x mode),
          SiLU evict (ACT) into persistent SBUF: rollkq (chan-major k|q
          interleaved per 128-token chunk) and vwin (guarded chan-major v).
  phase B prologue: beta + gate columns from rows scratch (one Sigmoid /
          Exp table load each).
  phase B (per 128-token chunk): PE transposes (fp16 PSUM), l2norm stats,
          UT-transform T^T via Neumann product, chunk-local prepass,
          serial scan (u = u0 - w S, o = q S + attn u, S += k^T u) with
          fp32 S master + fp16 S copy; FIR-long diag matmuls + FIR-short
          STT chains interleaved per tile to fill PE gaps.
  phase C (per chunk, pipelined one tile behind): FIR transposes,
          hierarchical gating (DVE fp16), RMSNorm, output projection.
"""
import numpy as np
import ml_dtypes
from contextlib import ExitStack

import concourse.bass as bass
import concourse.tile as tile
from concourse import bacc, mybir
from concourse.bass_utils import run_bass_kernel_spmd

F32 = mybir.dt.float32
F16 = mybir.dt.float16
AF = mybir.ActivationFunctionType
ALU = mybir.AluOpType

B, D, H, DK, DV = 2, 1024, 4, 256, 256
CONV_K, FIR_S, FIR_L = 4, 3, 31
CH = 128          # scan chunk (token tile)
NTILE = 512       # column tile for projections / FIR
P = 128
KT = D // P       # 8 contraction tiles over D
GUARD = CONV_K - 1
EPS_RMS = 1e-5
FGUARD = FIR_L    # guard cols ahead of token 0 in vwin


def build(L=4096):
    NT = L // NTILE
    NCH = L // CH
    CPN = NTILE // CH  # chunks per n-tile (4)

    nc = bacc.Bacc("TRN2", target_bir_lowering=False, debug=False, num_devices=8)

    xT_d = nc.dram_tensor("xT", [D, L], F16, kind="ExternalInput").ap()
    wq_d = nc.dram_tensor("wq", [D, DK], F16, kind="ExternalInput").ap()
    wk_d = nc.dram_tensor("wk", [D, DK], F16, kind="ExternalInput").ap()
    wv_d = nc.dram_tensor("wv", [D, DV], F16, kind="ExternalInput").ap()
    wsm_d = nc.dram_tensor("wsm", [D, 5], F16, kind="ExternalInput").ap()
    bias5_d = nc.dram_tensor("bias5", [5], F32, kind="ExternalInput").ap()
    # conv taps per (tensor, pt): [3, 2, 128, 4]
    ctaps_d = nc.dram_tensor("ctaps", [3, 2, P, CONV_K], F32, kind="ExternalInput").ap()
    # fir long-residual diagonal matrices: [pt=2, 31, 128, 128]
    fdiag_d = nc.dram_tensor("fdiag", [2, FIR_L, P, P], F16, kind="ExternalInput").ap()
    # fir short-residual taps: [2, 128, 3]
    staps_d = nc.dram_tensor("staps", [2, P, FIR_S], F32, kind="ExternalInput").ap()
    wo_d = nc.dram_tensor("wo", [DV, D], F16, kind="ExternalInput").ap()
    ident_d = nc.dram_tensor("ident", [P, P], F16, kind="ExternalInput").ap()
    masklt_d = nc.dram_tensor("masklt", [P, P], F16, kind="ExternalInput").ap()  # strict lower
    maskut_d = nc.dram_tensor("maskut", [P, P], F16, kind="ExternalInput").ap()  # upper incl diag
    out_d = nc.dram_tensor("out", [L, D], F16, kind="ExternalOutput").ap()

    with tile.TileContext(nc) as tc, ExitStack() as ctx:
        # ---------------- pools ----------------
        const = ctx.enter_context(tc.tile_pool(name="const", bufs=1))
        bigw = ctx.enter_context(tc.tile_pool(name="bigw", bufs=1))
        persist = ctx.enter_context(tc.tile_pool(name="persist", bufs=1))
        xtp = ctx.enter_context(tc.tile_pool(name="xtp", bufs=2))
        prep = ctx.enter_context(tc.tile_pool(name="prep", bufs=1))
        colp = ctx.enter_context(tc.tile_pool(name="colp", bufs=1))
        chk = ctx.enter_context(tc.tile_pool(name="chk", bufs=1))
        sp = ctx.enter_context(tc.tile_pool(name="sp", bufs=1))
        gat = ctx.enter_context(tc.tile_pool(name="gat", bufs=1))
        dram = ctx.enter_context(tc.tile_pool(name="dram", bufs=1, space="DRAM"))
        ps_big = ctx.enter_context(tc.tile_pool(name="ps_big", bufs=3, space="PSUM"))
        ps_med = ctx.enter_context(tc.tile_pool(name="ps_med", bufs=3, space="PSUM"))
        ps_t = ctx.enter_context(tc.tile_pool(name="ps_t", bufs=2, space="PSUM"))

        # ---------------- DRAM scratch ----------------
        rows_d = dram.tile([5, L], F32, name="rows_sc")

        # ---------------- constants / weights ----------------
        wq = bigw.tile([P, KT, DK], F16, tag="wq")
        nc.sync.dma_start(wq[:], wq_d.rearrange("(kt p) m -> p kt m", p=P))
        wk = bigw.tile([P, KT, DK], F16, tag="wk")
        nc.sync.dma_start(wk[:], wk_d.rearrange("(kt p) m -> p kt m", p=P))
        wv = bigw.tile([P, KT, DV], F16, tag="wv")
        nc.sync.dma_start(wv[:], wv_d.rearrange("(kt p) m -> p kt m", p=P))
        wsm = const.tile([P, KT, 5], F16)
        nc.sync.dma_start(wsm[:], wsm_d.rearrange("(kt p) m -> p kt m", p=P))
        ident = const.tile([P, P], F16)
        nc.sync.dma_start(ident[:], ident_d)
        masklt = const.tile([P, P], F16)
        nc.sync.dma_start(masklt[:], masklt_d)
        maskut = const.tile([P, P], F16)
        nc.sync.dma_start(maskut[:], maskut_d)
        bias5 = const.tile([5, 1], F32)
        nc.sync.dma_start(bias5[:], bias5_d.rearrange("(m o) -> m o", o=1))
        ctaps = const.tile([P, 3, 2, CONV_K], F32, name="ctaps")
        nc.sync.dma_start(ctaps[:], ctaps_d.rearrange("t pt p j -> p t pt j"))
        staps = const.tile([P, 2, FIR_S], F32, name="staps")
        nc.sync.dma_start(staps[:], staps_d.rearrange("pt p j -> p pt j"))
        fdiag = bigw.tile([P, 2, FIR_L, P], F16, tag="fdiag", name="fdiag")
        nc.sync.dma_start(fdiag[:], fdiag_d.rearrange("pt j p q -> p pt j q"))
        wo = bigw.tile([P, 2, D], F16, tag="wo", name="wo")
        nc.sync.dma_start(wo[:], wo_d.rearrange("(kt p) m -> p kt m", p=P))
        zeros3 = const.tile([P, GUARD], F16)
        nc.vector.memset(zeros3[:], 0.0)
        eps_l2 = const.tile([P, 1], F32)
        nc.vector.memset(eps_l2[:], 1e-6)
        eps_rms = const.tile([P, 1], F32)
        nc.vector.memset(eps_rms[:], EPS_RMS)

        # ---------------- persistent SBUF state ----------------
        # chan-major post-silu k|q interleaved per chunk: [P, pt, chunk, (k|q), CH]
        rollkq = persist.tile([P, 2, NCH, 2, CH], F16, name="rollkq")
        # chan-major post-silu v with FGUARD leading guard cols
        vwin = persist.tile([P, 2, FGUARD + L], F16, name="vwin")
        nc.vector.memset(vwin[:, :, 0:FGUARD], 0.0)
        # per-chunk outputs of the scan + beta-scaled v (token-major)
        o_all = persist.tile([P, NCH, DV], F16, name="o_all")
        vb_all = persist.tile([P, NCH, DV], F16, name="vb_all")

        S_sb = [sp.tile([P, DV], F32, name=f"S{kt}") for kt in range(2)]
        S16 = [sp.tile([P, DV], F16, name=f"S16_{kt}") for kt in range(2)]
        for kt in range(2):
            nc.vector.memset(S_sb[kt][:], 0.0)

        TENS = ("q", "k", "v")
        W_OF = {"q": wq, "k": wk, "v": wv}
        prev_pre = {}

        # ================= phase A: projections + conv + silu =================
        def phaseA(n):
            xt = xtp.tile([P, KT, NTILE], F16, tag="xt", name="xt")
            nc.sync.dma_start(xt[:], xT_d[:, n * NTILE:(n + 1) * NTILE]
                              .rearrange("(kt p) t -> p kt t", p=P))
            for t in TENS:
                for pt in range(2):
                    ps = ps_big.tile([P, NTILE], F32, tag="psb", name=f"ps_{t}{pt}")
                    for kt in range(KT):
                        nc.tensor.matmul(ps[:], W_OF[t][:, kt, pt * P:(pt + 1) * P],
                                         xt[:, kt, :], start=(kt == 0), stop=(kt == KT - 1))
                    key = f"pre{t}{pt}"
                    pre = prep.tile([P, GUARD + NTILE], F16, tag=key, bufs=2, name=key)
                    if n == 0:
                        nc.scalar.copy(pre[:, 0:GUARD], zeros3[:])
                    else:
                        nc.scalar.copy(pre[:, 0:GUARD], prev_pre[key][:, NTILE:NTILE + GUARD])
                    nc.scalar.copy(pre[:, GUARD:], ps[:])
                    prev_pre[key] = pre
                    # conv: 4-tap STT chain, all-fp16 SBUF (4x DVE mode)
                    ti = TENS.index(t)
                    acc = prep.tile([P, NTILE], F16, tag="cacc", bufs=2, name="cacc")
                    nc.vector.tensor_scalar_mul(acc[:], pre[:, 0:NTILE], ctaps[:, ti, pt, 0:1])
                    for j in range(1, CONV_K):
                        nc.vector.scalar_tensor_tensor(acc[:], pre[:, j:j + NTILE],
                                                       ctaps[:, ti, pt, j:j + 1], acc[:],
                                                       op0=ALU.mult, op1=ALU.add)
                    # silu -> persistent layout
                    if t == "v":
                        nc.scalar.activation(
                            vwin[:, pt, FGUARD + n * NTILE: FGUARD + (n + 1) * NTILE],
                            acc[:], AF.Silu)
                    else:
                        koff = 0 if t == "k" else 1
                        nc.scalar.activation(
                            rollkq[:, pt, n * CPN:(n + 1) * CPN, koff, :],
                            acc[:], AF.Silu)
            # small projections [5, NTILE]
            ps5 = ps_big.tile([P, NTILE], F32, tag="psb", name="ps5")
            for kt in range(KT):
                nc.tensor.matmul(ps5[:5, :], wsm[:, kt, :], xt[:, kt, :],
                                 start=(kt == 0), stop=(kt == KT - 1))
            rows_sb = colp.tile([5, NTILE], F32, tag="rows_sb", bufs=2, name="rows_sb")
            nc.scalar.activation(rows_sb[:], ps5[:5, :], AF.Identity, bias=bias5[:])
            nc.sync.dma_start(rows_d[:, n * NTILE:(n + 1) * NTILE], rows_sb[:])

        for n in range(NT):
            phaseA(n)

        # ================= phase B prologue: beta + gate columns =================
        def col_from_row(r, name):
            t = colp.tile([P, NCH], F32, tag=name, bufs=1, name=name)
            nc.sync.dma_start(t[:], rows_d[r].rearrange("(nn p) -> p nn", p=P))
            return t

        beta_all = colp.tile([P, NCH], F32, tag="beta_all", bufs=1, name="beta_all")
        braw = col_from_row(0, "braw")
        nc.scalar.activation(beta_all[:], braw[:], AF.Sigmoid)
        wg_c = colp.tile([P, NCH], F32, tag="wg_c", bufs=1, name="wg_c")
        wgraw = col_from_row(1, "wgraw")
        nc.scalar.activation(wg_c[:], wgraw[:], AF.Sigmoid)
        l0_c = col_from_row(2, "l0_c")
        l1_c = col_from_row(3, "l1_c")
        l2_c = col_from_row(4, "l2_c")
        mx = colp.tile([P, NCH], F32, tag="mx", bufs=1, name="mx")
        nc.vector.tensor_tensor(mx[:], l0_c[:], l1_c[:], op=ALU.max)
        nc.vector.tensor_tensor(mx[:], mx[:], l2_c[:], op=ALU.max)
        e0 = colp.tile([P, NCH], F32, tag="e0", bufs=1, name="e0")
        e1 = colp.tile([P, NCH], F32, tag="e1", bufs=1, name="e1")
        e2 = colp.tile([P, NCH], F32, tag="e2", bufs=1, name="e2")
        for src, dst in ((l0_c, e0), (l1_c, e1), (l2_c, e2)):
            nc.vector.tensor_tensor(dst[:], src[:], mx[:], op=ALU.subtract)
            nc.scalar.activation(dst[:], dst[:], AF.Exp)
        esum = colp.tile([P, NCH], F32, tag="esum", bufs=1, name="esum")
        nc.vector.tensor_tensor(esum[:], e0[:], e1[:], op=ALU.add)
        nc.vector.tensor_tensor(esum[:], esum[:], e2[:], op=ALU.add)
        erec = colp.tile([P, NCH], F32, tag="erec", bufs=1, name="erec")
        nc.vector.reciprocal(erec[:], esum[:])
        p1 = colp.tile([P, NCH], F32, tag="p1", bufs=1, name="p1")
        p2 = colp.tile([P, NCH], F32, tag="p2", bufs=1, name="p2")
        for src, dst in ((e1, p1), (e2, p2)):
            nc.vector.tensor_tensor(dst[:], src[:], erec[:], op=ALU.mult)
        brec = colp.tile([P, NCH], F32, tag="brec", bufs=1, name="brec")
        nc.vector.reciprocal(brec[:], beta_all[:])
        wg1m = colp.tile([P, NCH], F32, tag="wg1m", bufs=1, name="wg1m")
        nc.vector.tensor_scalar(wg1m[:], wg_c[:], -1.0, 1.0, op0=ALU.mult, op1=ALU.add)

        # ================= phase B: per-chunk scan =================
        def vtok_col(c):
            return vwin[:, :, FGUARD + c * CH: FGUARD + (c + 1) * CH]

        def norms(c):
            beta_c = beta_all[:, c:c + 1]
            res = {"beta": beta_c}
            # v: transpose + beta-scale -> vb_all[c]
            tpv = ps_t.tile([P, 2, CH], F16, tag="pst", name="tp_v")
            for pt in range(2):
                nc.tensor.transpose(tpv[:, pt, :],
                                    vwin[:, pt, FGUARD + c * CH: FGUARD + (c + 1) * CH],
                                    ident[:])
            nc.vector.tensor_scalar_mul(vb_all[:, c, :], tpv[:], beta_c)
            res["kTsl"] = [rollkq[:, pt, c, 0, :] for pt in range(2)]
            res["qTsl"] = [rollkq[:, pt, c, 1, :] for pt in range(2)]
            res["kqTsl"] = [rollkq[:, pt, c, :, :] for pt in range(2)]
            # token-major q/k for l2 stats
            toks = {}
            for t, koff in (("q", 1), ("k", 0)):
                tok = chk.tile([P, DV], F16, tag=f"{t}tok", bufs=2, name=f"{t}tok")
                tpt = ps_t.tile([P, 2, CH], F16, tag="pst", name=f"tp_{t}")
                for pt in range(2):
                    nc.tensor.transpose(tpt[:, pt, :], rollkq[:, pt, c, koff, :], ident[:])
                nc.scalar.copy(tok[:], tpt[:])
                toks[t] = tok
            ssqs = {}
            for t in ("q", "k"):
                scr = chk.tile([P, DV], F16, tag="sq_scr", bufs=2, name="sq_scr")
                ssq = chk.tile([P, 1], F32, tag=f"ssq{t}", bufs=2, name=f"ssq{t}")
                nc.vector.scalar_tensor_tensor(scr[:], toks[t][:], 1.0, toks[t][:],
                                               op0=ALU.mult, op1=ALU.mult, accum_out=ssq[:])
                ssqs[t] = ssq
            sroots = {}
            for t in ("q", "k"):
                sroot = chk.tile([P, 1], F32, tag=f"sroot{t}", bufs=2, name=f"sroot{t}")
                nc.scalar.activation(sroot[:], ssqs[t][:], AF.Sqrt, bias=eps_l2[:])
                sroots[t] = sroot
            for t in ("q", "k"):
                rinv = chk.tile([P, 1], F32, tag=f"rinv{t}", bufs=3, name=f"rinv{t}")
                nc.vector.reciprocal(rinv[:], sroots[t][:])
                res["rinv" + t] = rinv
            khat = chk.tile([P, DV], F16, tag="khat", bufs=2, name="khat")
            nc.vector.tensor_scalar_mul(khat[:], toks["k"][:], res["rinvk"][:])
            res["khat"] = khat
            khatb = chk.tile([P, DV], F16, tag="khatb", bufs=2, name="khatb")
            nc.vector.tensor_scalar_mul(khatb[:], khat[:], beta_c)
            res["khatb"] = khatb
            return res

        def mm_small(lhsT, rhs, name, evict_eng):
            ps = ps_med.tile([P, DV], F32, tag="psm", name=f"ps_{name}")
            nc.tensor.matmul(ps[:, :P], lhsT, rhs, start=True, stop=True)
            sb = chk.tile([P, P], F16, tag=name, bufs=1, name=name)
            if evict_eng == "v":
                nc.vector.tensor_copy(sb[:], ps[:, :P])
            else:
                nc.scalar.copy(sb[:], ps[:, :P])
            return sb

        def prepass(c, nr):
            beta_c = nr["beta"]
            rinvk, rinvq = nr["rinvk"], nr["rinvq"]
            # [Graw | Braw] = kraw @ [kraw | qraw]^T in one N=256 stream per pt
            gps = ps_med.tile([P, DV], F32, tag="psm", name="gps")
            for pt in range(2):
                nc.tensor.matmul(gps[:], nr["kTsl"][pt], nr["kqTsl"][pt],
                                 start=(pt == 0), stop=(pt == 1))
            # N1 = tril_strict * rowscale_{beta*rinvk}(Graw); Mt = N1^T rowscale rinvk
            br = chk.tile([P, 1], F32, tag="br", bufs=2, name="br")
            nc.vector.tensor_tensor(br[:], beta_c, rinvk[:], op=ALU.mult)
            N1 = chk.tile([P, P], F16, tag="N1", bufs=2, name="N1")
            nc.vector.scalar_tensor_tensor(N1[:], gps[:, :P], br[:], masklt[:],
                                           op0=ALU.mult, op1=ALU.mult)
            mps = ps_t.tile([P, 2, CH], F16, tag="pst", name="mps")
            nc.tensor.transpose(mps[:, 0, :], N1[:], ident[:])
            Mt = chk.tile([P, P], F16, tag="Mt", bufs=1, name="Mt")
            nc.vector.tensor_scalar_mul(Mt[:], mps[:, 0, :], rinvk[:])
            nps = ps_t.tile([P, 2, CH], F16, tag="pst", name="nps")
            nc.tensor.transpose(nps[:, 0, :], Mt[:], ident[:])
            Nt = chk.tile([P, P], F16, tag="Nt", bufs=1, name="Nt")
            nc.scalar.copy(Nt[:], nps[:, 0, :])
            # powers
            N2 = mm_small(Mt[:], Nt[:], "N2", "v")
            M2 = mm_small(Nt[:], Mt[:], "M2", "s")
            N4 = mm_small(M2[:], N2[:], "N4", "v")
            M4 = mm_small(N2[:], M2[:], "M4", "s")
            N8 = mm_small(M4[:], N4[:], "N8", "v")
            M8 = mm_small(N4[:], M4[:], "M8", "s")
            N16 = mm_small(M8[:], N8[:], "N16", "v")
            # P chain: P1 = I - Mt ; P_{j+1} = P_j + Npow^T @ P_j
            P1 = chk.tile([P, P], F16, tag="P1", bufs=1, name="P1")
            nc.vector.tensor_tensor(P1[:], ident[:], Mt[:], op=ALU.subtract)
            Pc = P1
            for Npow, nm in ((N2, "P2"), (N4, "P3"), (N8, "P4"), (N16, "TTt")):
                pps = ps_med.tile([P, DV], F32, tag="psm", name=f"pps_{nm}")
                nc.tensor.matmul(pps[:, :P], Npow[:], Pc[:], start=True, stop=True)
                nxt = chk.tile([P, P], F16, tag=nm, bufs=1 if nm != "TTt" else 2, name=nm)
                nc.vector.tensor_tensor(nxt[:], Pc[:], pps[:, :P], op=ALU.add)
                Pc = nxt
            TTt = Pc
            negTT = chk.tile([P, P], F16, tag="negTT", bufs=2, name="negTT")
            nc.vector.tensor_scalar_mul(negTT[:], TTt[:], -1.0)
            # attn^T = rowscale_{rinvk}(triu_incl * Braw)
            attnT = chk.tile([P, P], F16, tag="attnT", bufs=2, name="attnT")
            nc.vector.scalar_tensor_tensor(attnT[:], gps[:, P:], rinvk[:], maskut[:],
                                           op0=ALU.mult, op1=ALU.mult)
            # w^T(neg): [128, 2, 128]
            wTn = chk.tile([P, 2, CH], F16, tag="wTn", bufs=2, name="wTn")
            for kt in range(2):
                wps = ps_med.tile([P, DV], F32, tag="psm", name="wps")
                nc.tensor.matmul(wps[:, :P], nr["khatb"][:, kt * P:(kt + 1) * P], negTT[:],
                                 start=True, stop=True)
                nc.scalar.copy(wTn[:, kt, :], wps[:, :P])
            # u0 = T @ vb : lhsT = T^T
            ups = ps_med.tile([P, DV], F32, tag="psm", name="ups")
            nc.tensor.matmul(ups[:], TTt[:], vb_all[:, c, :], start=True, stop=True)
            u0 = chk.tile([P, DV], F16, tag="u0", bufs=2, name="u0")
            nc.scalar.copy(u0[:], ups[:])
            return {"TTt": TTt, "attnT": attnT, "wTn": wTn, "u0": u0,
                    "qTsl": nr["qTsl"], "khat": nr["khat"], "rinvq": rinvq}

        def serial(c, pr):
            if c == 0:
                u16 = pr["u0"]
            else:
                ups = ps_med.tile([P, DV], F32, tag="psm", name="ups_s")
                for kt in range(2):
                    nc.tensor.matmul(ups[:], pr["wTn"][:, kt, :], S16[kt][:],
                                     start=(kt == 0), stop=(kt == 1))
                u16 = chk.tile([P, DV], F16, tag="u16", bufs=2, name="u16")
                nc.vector.tensor_tensor(u16[:], ups[:], pr["u0"][:], op=ALU.add)
            # o
            ops = ps_med.tile([P, DV], F32, tag="psm", name="ops")
            if c == 0:
                nc.tensor.matmul(ops[:], pr["attnT"][:], u16[:], start=True, stop=True)
            else:
                for kt in range(2):
                    nc.tensor.matmul(ops[:], pr["qTsl"][kt], S16[kt][:],
                                     start=(kt == 0), stop=False)
                nc.tensor.matmul(ops[:], pr["attnT"][:], u16[:], start=False, stop=True)
            nc.vector.tensor_scalar_mul(o_all[:, c, :], ops[:], pr["rinvq"][:])
            # S += k^T u
            for kt in range(2):
                dps = ps_med.tile([P, DV], F32, tag="psm", name=f"dps{kt}")
                nc.tensor.matmul(dps[:], pr["khat"][:, kt * P:(kt + 1) * P], u16[:],
                                 start=True, stop=True)
                if c == 0:
                    nc.vector.tensor_copy(S_sb[kt][:], dps[:])
                else:
                    nc.vector.tensor_tensor(S_sb[kt][:], S_sb[kt][:], dps[:], op=ALU.add)
                nc.scalar.copy(S16[kt][:], S_sb[kt][:])

        # FIR long (PE diag matmuls) + short (DVE STT) for one n-tile
        def fir_tile(n):
            fch = {}
            for pt in range(2):
                ps = ps_big.tile([P, NTILE], F32, tag="psb", name="ps_ll")
                for j in range(FIR_L):
                    nc.tensor.matmul(ps[:], fdiag[:, pt, j, :],
                                     vwin[:, pt, n * NTILE + 1 + j: n * NTILE + 1 + j + NTILE],
                                     start=(j == 0), stop=(j == FIR_L - 1))
                ll = gat.tile([P, NTILE], F16, tag="llch", bufs=2, name="llch")
                nc.scalar.copy(ll[:], ps[:])
                fch[("ll", pt)] = ll
                ls = gat.tile([P, NTILE], F16, tag="lsch", bufs=2, name="lsch")
                bs = FGUARD - FIR_S + 1 + n * NTILE
                nc.vector.tensor_scalar_mul(ls[:], vwin[:, pt, bs:bs + NTILE], staps[:, pt, 0:1])
                for j in range(1, FIR_S):
                    nc.vector.scalar_tensor_tensor(ls[:], vwin[:, pt, bs + j:bs + j + NTILE],
                                                   staps[:, pt, j:j + 1], ls[:],
                                                   op0=ALU.mult, op1=ALU.add)
                fch[("ls", pt)] = ls
            return fch

        # ================= phase C: gating + output projection =================
        def gate_out(lt, fch):
            off = (lt % CPN) * CH
            cs = lambda t: t[:, lt:lt + 1]
            toks = {}
            for f in ("ls", "ll"):
                tokt = gat.tile([P, DV], F16, tag=f"{f}tok", bufs=2, name=f"{f}tok")
                tp = ps_t.tile([P, 2, CH], F16, tag="pst", name=f"tp_{f}")
                for pt in range(2):
                    nc.tensor.transpose(tp[:, pt, :], fch[(f, pt)][:, off:off + CH], ident[:])
                nc.scalar.copy(tokt[:], tp[:])
                toks[f] = tokt
            mix = gat.tile([P, DV], F16, tag="gtmp", bufs=6, name="mix")
            nc.vector.tensor_scalar_mul(mix[:], vb_all[:, lt, :], cs(brec))
            mix2 = gat.tile([P, DV], F16, tag="gtmp", bufs=6, name="mix2")
            nc.vector.scalar_tensor_tensor(mix2[:], toks["ls"][:], cs(p1), mix[:],
                                           op0=ALU.mult, op1=ALU.add)
            mix3 = gat.tile([P, DV], F16, tag="gtmp", bufs=6, name="mix3")
            nc.vector.scalar_tensor_tensor(mix3[:], toks["ll"][:], cs(p2), mix2[:],
                                           op0=ALU.mult, op1=ALU.add)
            tmp = gat.tile([P, DV], F16, tag="gtmp", bufs=6, name="tmpg")
            nc.vector.tensor_scalar_mul(tmp[:], mix3[:], cs(wg1m))
            om = gat.tile([P, DV], F16, tag="gtmp", bufs=6, name="om")
            nc.vector.scalar_tensor_tensor(om[:], o_all[:, lt, :], cs(wg_c), tmp[:],
                                           op0=ALU.mult, op1=ALU.add)
            scr = gat.tile([P, DV], F16, tag="scr_g", bufs=2, name="scr_g")
            ssq = gat.tile([P, 1], F32, tag="ssq_g", bufs=2, name="ssq_g")
            nc.vector.scalar_tensor_tensor(scr[:], om[:], 1.0, om[:],
                                           op0=ALU.mult, op1=ALU.mult, accum_out=ssq[:])
            srt = gat.tile([P, 1], F32, tag="srt_g", bufs=2, name="srt_g")
            nc.scalar.activation(srt[:], ssq[:], AF.Sqrt, bias=eps_rms[:], scale=1.0 / DV)
            rin = gat.tile([P, 1], F32, tag="rin_g", bufs=2, name="rin_g")
            nc.vector.reciprocal(rin[:], srt[:])
            on = gat.tile([P, DV], F16, tag="on_g", bufs=2, name="on_g")
            nc.vector.tensor_scalar_mul(on[:], om[:], rin[:])
            # output projection
            onT = gat.tile([P, 2, CH], F16, tag="onT", bufs=2, name="onT")
            tp = ps_t.tile([P, 2, CH], F16, tag="pst", name="tp_on")
            for pt in range(2):
                nc.tensor.transpose(tp[:, pt, :], on[:, pt * P:(pt + 1) * P], ident[:])
            nc.scalar.copy(onT[:], tp[:])
            out_sb = gat.tile([P, D], F16, tag="out_sb", bufs=2, name="out_sb")
            for nt2 in range(2):
                ops = ps_big.tile([P, NTILE], F32, tag="psb", name="ops_o")
                for kt in range(2):
                    nc.tensor.matmul(ops[:], onT[:, kt, :],
                                     wo[:, kt, nt2 * NTILE:(nt2 + 1) * NTILE],
                                     start=(kt == 0), stop=(kt == 1))
                nc.scalar.copy(out_sb[:, nt2 * NTILE:(nt2 + 1) * NTILE], ops[:])
            nc.sync.dma_start(out_d[lt * CH:(lt + 1) * CH, :], out_sb[:])

        # ---- emit phases B & C interleaved (C one tile behind) ----
        pending = None       # (c, prepass result)
        pend_tile = None     # (lts, fch)
        for n in range(NT):
            fch = fir_tile(n)
            for c in range(n * CPN, (n + 1) * CPN):
                nr = norms(c)
                pr = prepass(c, nr)
                if pending is not None:
                    serial(pending[0], pending[1])
                pending = (c, pr)
            if pend_tile is not None:
                for lt in pend_tile[0]:
                    gate_out(lt, pend_tile[1])
            pend_tile = (list(range(n * CPN, (n + 1) * CPN)), fch)
        serial(pending[0], pending[1])
        for lt in pend_tile[0]:
            gate_out(lt, pend_tile[1])

    nc.compile()
    return nc


# ---------------- host side ----------------

def _diag_stack(taps):
    """taps [C, K] -> [2, K, 128, 128] diag matrices."""
    C, K = taps.shape
    out = np.zeros((2, K, P, P), np.float32)
    for pt in range(2):
        for j in range(K):
            np.fill_diagonal(out[pt, j], taps[pt * P:(pt + 1) * P, j])
    return out


def make_core_inputs(inputs, b, h, L):
    f = lambda a: np.ascontiguousarray(np.asarray(a, np.float32))
    x = f(inputs['hidden_states'])[b]          # [L, D]
    temp = float(np.exp(np.asarray(inputs['log_temp'], np.float64)[h]))
    wsm = np.concatenate([
        f(inputs['Wb'])[:, h:h + 1],
        f(inputs['Wg'])[:, h:h + 1],
        f(inputs['Wl'])[:, 3 * h:3 * h + 3] / temp], axis=1)
    bias5 = np.array([0.0, float(np.asarray(inputs['bg'], np.float64)[h]),
                      *(np.asarray(inputs['bl'], np.float64)[3 * h:3 * h + 3] / temp)],
                     np.float32)
    ct = np.stack([
        f(inputs['conv_q'])[h * DK:(h + 1) * DK].reshape(2, P, CONV_K),
        f(inputs['conv_k'])[h * DK:(h + 1) * DK].reshape(2, P, CONV_K),
        f(inputs['conv_v'])[h * DV:(h + 1) * DV].reshape(2, P, CONV_K)])  # [3, 2, 128, 4]
    # residual FIR taps: fir = delta + r  ->  local = v + FIR_r(v); softmax sums to 1
    fs = f(inputs['fir_short'])[h].copy()   # [DV, 3]
    fs[:, -1] -= 1.0
    fl = f(inputs['fir_long'])[h].copy()    # [DV, 31]
    fl[:, -1] -= 1.0
    fd = _diag_stack(fl).astype(np.float16)
    st = fs.reshape(2, P, FIR_S)
    wo = f(inputs['rms_w'])[:, None] * f(inputs['Wo'])[h * DV:(h + 1) * DV]
    h16 = np.float16
    return dict(
        xT=np.ascontiguousarray(x.T).astype(h16),
        wq=np.ascontiguousarray(f(inputs['Wq'])[:, h * DK:(h + 1) * DK]).astype(h16),
        wk=np.ascontiguousarray(f(inputs['Wk'])[:, h * DK:(h + 1) * DK]).astype(h16),
        wv=np.ascontiguousarray(f(inputs['Wv'])[:, h * DV:(h + 1) * DV]).astype(h16),
        wsm=wsm.astype(h16), bias5=bias5,
        ctaps=ct, fdiag=fd, staps=st.astype(np.float32), wo=wo.astype(h16),
        ident=np.eye(P, dtype=h16),
        masklt=np.tril(np.ones((P, P), h16), -1),
        maskut=np.triu(np.ones((P, P), h16), 0),
    )


_NC_CACHE = {}


def _get_nc(L):
    if L not in _NC_CACHE:
        _NC_CACHE[L] = build(L)
    return _NC_CACHE[L]


def kernel(**inputs):
    x = np.asarray(inputs['hidden_states'])
    Bx, L, _ = x.shape
    nc = _get_nc(L)
    in_maps = [make_core_inputs(inputs, c // H, c % H, L) for c in range(8)]
    res = run_bass_kernel_spmd(nc, in_maps, core_ids=list(range(8)))
    out = np.zeros((Bx, L, D), np.float32)
    for c in range(8):
        out[c // H] += res.results[c]['out'].astype(np.float32)
    return out


# revision 18
# speedup vs baseline: 1.5970x; 1.0565x over previous
"""DeltaNet Bass kernel for Trainium2, 8-core SPMD — fp16 matmul pipeline.

Sharding: core = (b, h) for b in 0..1, h in 0..3  (b*4 + h).
Each core computes the full per-(batch,head) pipeline and its partial
output projection out_partial[L, D] in fp16; the host sums the 4
head-partials per batch.

Device pipeline per core (PSUM accumulate fp32, operands fp16):
  phase A (per 512-col tile, all 8 tiles first): q/k/v/small projections
          (PE, fp16), causal 4-tap conv (DVE STT chain, fp16 4x mode),
          SiLU evict (ACT) into persistent SBUF: rollkq (chan-major k|q
          interleaved per 128-token chunk) and vwin (guarded chan-major v).
  phase B prologue: beta + gate columns from rows scratch (one Sigmoid /
          Exp table load each).
  phase B (per 128-token chunk): PE transposes (fp16 PSUM), l2norm stats,
          UT-transform T^T via Neumann product, chunk-local prepass,
          serial scan (u = u0 - w S, o = q S + attn u, S += k^T u) with
          fp32 S master + fp16 S copy; FIR-long diag matmuls + FIR-short
          STT chains interleaved per tile to fill PE gaps.
  phase C (per chunk, pipelined one tile behind): FIR transposes,
          hierarchical gating (DVE fp16), RMSNorm, output projection.
"""
import numpy as np
import ml_dtypes
from contextlib import ExitStack

import concourse.bass as bass
import concourse.tile as tile
from concourse import bacc, mybir
from concourse.bass_utils import run_bass_kernel_spmd

F32 = mybir.dt.float32
F16 = mybir.dt.float16
AF = mybir.ActivationFunctionType
ALU = mybir.AluOpType

B, D, H, DK, DV = 2, 1024, 4, 256, 256
CONV_K, FIR_S, FIR_L = 4, 3, 31
CH = 128          # scan chunk (token tile)
NTILE = 512       # column tile for projections / FIR
P = 128
KT = D // P       # 8 contraction tiles over D
GUARD = CONV_K - 1
EPS_RMS = 1e-5
FGUARD = FIR_L    # guard cols ahead of token 0 in vwin


def build(L=4096):
    NT = L // NTILE
    NCH = L // CH
    CPN = NTILE // CH  # chunks per n-tile (4)

    nc = bacc.Bacc("TRN2", target_bir_lowering=False, debug=False, num_devices=8)

    xT_d = nc.dram_tensor("xT", [D, L], F16, kind="ExternalInput").ap()
    wq_d = nc.dram_tensor("wq", [D, DK], F16, kind="ExternalInput").ap()
    wk_d = nc.dram_tensor("wk", [D, DK], F16, kind="ExternalInput").ap()
    wv_d = nc.dram_tensor("wv", [D, DV], F16, kind="ExternalInput").ap()
    wsm_d = nc.dram_tensor("wsm", [D, 5], F16, kind="ExternalInput").ap()
    bias5_d = nc.dram_tensor("bias5", [5], F32, kind="ExternalInput").ap()
    # conv taps per (tensor, pt): [3, 2, 128, 4]
    ctaps_d = nc.dram_tensor("ctaps", [3, 2, P, CONV_K], F32, kind="ExternalInput").ap()
    # fir long-residual diagonal matrices: [pt=2, 31, 128, 128]
    fdiag_d = nc.dram_tensor("fdiag", [2, FIR_L, P, P], F16, kind="ExternalInput").ap()
    # fir short-residual taps: [2, 128, 3]
    staps_d = nc.dram_tensor("staps", [2, P, FIR_S], F32, kind="ExternalInput").ap()
    wo_d = nc.dram_tensor("wo", [DV, D], F16, kind="ExternalInput").ap()
    ident_d = nc.dram_tensor("ident", [P, P], F16, kind="ExternalInput").ap()
    masklt_d = nc.dram_tensor("masklt", [P, P], F16, kind="ExternalInput").ap()  # strict lower
    maskut_d = nc.dram_tensor("maskut", [P, P], F16, kind="ExternalInput").ap()  # upper incl diag
    out_d = nc.dram_tensor("out", [L, D], F16, kind="ExternalOutput").ap()

    with tile.TileContext(nc) as tc, ExitStack() as ctx:
        # ---------------- pools ----------------
        const = ctx.enter_context(tc.tile_pool(name="const", bufs=1))
        bigw = ctx.enter_context(tc.tile_pool(name="bigw", bufs=1))
        persist = ctx.enter_context(tc.tile_pool(name="persist", bufs=1))
        xtp = ctx.enter_context(tc.tile_pool(name="xtp", bufs=2))
        prep = ctx.enter_context(tc.tile_pool(name="prep", bufs=1))
        colp = ctx.enter_context(tc.tile_pool(name="colp", bufs=1))
        chk = ctx.enter_context(tc.tile_pool(name="chk", bufs=1))
        sp = ctx.enter_context(tc.tile_pool(name="sp", bufs=1))
        gat = ctx.enter_context(tc.tile_pool(name="gat", bufs=1))
        dram = ctx.enter_context(tc.tile_pool(name="dram", bufs=1, space="DRAM"))
        ps_big = ctx.enter_context(tc.tile_pool(name="ps_big", bufs=3, space="PSUM"))
        ps_med = ctx.enter_context(tc.tile_pool(name="ps_med", bufs=3, space="PSUM"))
        ps_t = ctx.enter_context(tc.tile_pool(name="ps_t", bufs=2, space="PSUM"))

        # ---------------- DRAM scratch ----------------
        rows_d = dram.tile([5, L], F32, name="rows_sc")

        # ---------------- constants / weights ----------------
        wq = bigw.tile([P, KT, DK], F16, tag="wq")
        nc.sync.dma_start(wq[:], wq_d.rearrange("(kt p) m -> p kt m", p=P))
        wk = bigw.tile([P, KT, DK], F16, tag="wk")
        nc.sync.dma_start(wk[:], wk_d.rearrange("(kt p) m -> p kt m", p=P))
        wv = bigw.tile([P, KT, DV], F16, tag="wv")
        nc.sync.dma_start(wv[:], wv_d.rearrange("(kt p) m -> p kt m", p=P))
        wsm = const.tile([P, KT, 5], F16)
        nc.sync.dma_start(wsm[:], wsm_d.rearrange("(kt p) m -> p kt m", p=P))
        ident = const.tile([P, P], F16)
        nc.sync.dma_start(ident[:], ident_d)
        masklt = const.tile([P, P], F16)
        nc.sync.dma_start(masklt[:], masklt_d)
        maskut = const.tile([P, P], F16)
        nc.sync.dma_start(maskut[:], maskut_d)
        bias5 = const.tile([5, 1], F32)
        nc.sync.dma_start(bias5[:], bias5_d.rearrange("(m o) -> m o", o=1))
        ctaps = const.tile([P, 3, 2, CONV_K], F32, name="ctaps")
        nc.sync.dma_start(ctaps[:], ctaps_d.rearrange("t pt p j -> p t pt j"))
        staps = const.tile([P, 2, FIR_S], F32, name="staps")
        nc.sync.dma_start(staps[:], staps_d.rearrange("pt p j -> p pt j"))
        fdiag = bigw.tile([P, 2, FIR_L, P], F16, tag="fdiag", name="fdiag")
        nc.sync.dma_start(fdiag[:], fdiag_d.rearrange("pt j p q -> p pt j q"))
        wo = bigw.tile([P, 2, D], F16, tag="wo", name="wo")
        nc.sync.dma_start(wo[:], wo_d.rearrange("(kt p) m -> p kt m", p=P))
        zeros3 = const.tile([P, GUARD], F16)
        nc.vector.memset(zeros3[:], 0.0)
        eps_l2 = const.tile([P, 1], F32)
        nc.vector.memset(eps_l2[:], 1e-6)
        eps_rms = const.tile([P, 1], F32)
        nc.vector.memset(eps_rms[:], EPS_RMS)

        # ---------------- persistent SBUF state ----------------
        # chan-major post-silu k|q interleaved per chunk: [P, pt, chunk, (k|q), CH]
        rollkq = persist.tile([P, 2, NCH, 2, CH], F16, name="rollkq")
        # chan-major post-silu v with FGUARD leading guard cols
        vwin = persist.tile([P, 2, FGUARD + L], F16, name="vwin")
        nc.vector.memset(vwin[:, :, 0:FGUARD], 0.0)
        # per-chunk outputs of the scan + beta-scaled v (token-major)
        o_all = persist.tile([P, NCH, DV], F16, name="o_all")
        vb_all = persist.tile([P, NCH, DV], F16, name="vb_all")

        S16 = sp.tile([P, 2, DV], F16, name="S16")

        TENS = ("q", "k", "v")
        W_OF = {"q": wq, "k": wk, "v": wv}
        prev_pre = {}

        # ================= phase A: projections + conv + silu =================
        def phaseA(n):
            xt = xtp.tile([P, KT, NTILE], F16, tag="xt", name="xt")
            nc.sync.dma_start(xt[:], xT_d[:, n * NTILE:(n + 1) * NTILE]
                              .rearrange("(kt p) t -> p kt t", p=P))
            for t in TENS:
                for pt in range(2):
                    ps = ps_big.tile([P, NTILE], F32, tag="psb", name=f"ps_{t}{pt}")
                    for kt in range(KT):
                        nc.tensor.matmul(ps[:], W_OF[t][:, kt, pt * P:(pt + 1) * P],
                                         xt[:, kt, :], start=(kt == 0), stop=(kt == KT - 1))
                    key = f"pre{t}{pt}"
                    pre = prep.tile([P, GUARD + NTILE], F16, tag=key, bufs=2, name=key)
                    if n == 0:
                        nc.scalar.copy(pre[:, 0:GUARD], zeros3[:])
                    else:
                        nc.scalar.copy(pre[:, 0:GUARD], prev_pre[key][:, NTILE:NTILE + GUARD])
                    nc.scalar.copy(pre[:, GUARD:], ps[:])
                    prev_pre[key] = pre
                    # conv: 4-tap chain on DVE in TS+TT form (tensor_scalar and
                    # tensor_tensor hit the 2x 16-bit DVE mode; STT does not).
                    ti = TENS.index(t)
                    acc = prep.tile([P, NTILE], F16, tag="cacc", bufs=2, name="cacc")
                    nc.vector.tensor_scalar_mul(acc[:], pre[:, 0:NTILE],
                                                ctaps[:, ti, pt, 0:1])
                    for j in range(1, CONV_K):
                        prod = prep.tile([P, NTILE], F16, tag="cprod", bufs=2, name="cprod")
                        nc.vector.tensor_scalar_mul(prod[:], pre[:, j:j + NTILE],
                                                    ctaps[:, ti, pt, j:j + 1])
                        nc.vector.tensor_tensor(acc[:], acc[:], prod[:], op=ALU.add)
                    # silu -> persistent layout
                    if t == "v":
                        nc.scalar.activation(
                            vwin[:, pt, FGUARD + n * NTILE: FGUARD + (n + 1) * NTILE],
                            acc[:], AF.Silu)
                    else:
                        koff = 0 if t == "k" else 1
                        nc.scalar.activation(
                            rollkq[:, pt, n * CPN:(n + 1) * CPN, koff, :],
                            acc[:], AF.Silu)
            # small projections [5, NTILE]
            ps5 = ps_big.tile([P, NTILE], F32, tag="psb", name="ps5")
            for kt in range(KT):
                nc.tensor.matmul(ps5[:5, :], wsm[:, kt, :], xt[:, kt, :],
                                 start=(kt == 0), stop=(kt == KT - 1))
            rows_sb = colp.tile([5, NTILE], F32, tag="rows_sb", bufs=2, name="rows_sb")
            nc.scalar.activation(rows_sb[:], ps5[:5, :], AF.Identity, bias=bias5[:])
            nc.sync.dma_start(rows_d[:, n * NTILE:(n + 1) * NTILE], rows_sb[:])

        for n in range(NT):
            phaseA(n)

        # ================= phase B prologue: beta + gate columns =================
        def col_from_row(r, name):
            t = colp.tile([P, NCH], F32, tag=name, bufs=1, name=name)
            nc.sync.dma_start(t[:], rows_d[r].rearrange("(nn p) -> p nn", p=P))
            return t

        beta_all = colp.tile([P, NCH], F32, tag="beta_all", bufs=1, name="beta_all")
        braw = col_from_row(0, "braw")
        nc.scalar.activation(beta_all[:], braw[:], AF.Sigmoid)
        wg_c = colp.tile([P, NCH], F32, tag="wg_c", bufs=1, name="wg_c")
        wgraw = col_from_row(1, "wgraw")
        nc.scalar.activation(wg_c[:], wgraw[:], AF.Sigmoid)
        l0_c = col_from_row(2, "l0_c")
        l1_c = col_from_row(3, "l1_c")
        l2_c = col_from_row(4, "l2_c")
        mx = colp.tile([P, NCH], F32, tag="mx", bufs=1, name="mx")
        nc.vector.tensor_tensor(mx[:], l0_c[:], l1_c[:], op=ALU.max)
        nc.vector.tensor_tensor(mx[:], mx[:], l2_c[:], op=ALU.max)
        e0 = colp.tile([P, NCH], F32, tag="e0", bufs=1, name="e0")
        e1 = colp.tile([P, NCH], F32, tag="e1", bufs=1, name="e1")
        e2 = colp.tile([P, NCH], F32, tag="e2", bufs=1, name="e2")
        for src, dst in ((l0_c, e0), (l1_c, e1), (l2_c, e2)):
            nc.vector.tensor_tensor(dst[:], src[:], mx[:], op=ALU.subtract)
            nc.scalar.activation(dst[:], dst[:], AF.Exp)
        esum = colp.tile([P, NCH], F32, tag="esum", bufs=1, name="esum")
        nc.vector.tensor_tensor(esum[:], e0[:], e1[:], op=ALU.add)
        nc.vector.tensor_tensor(esum[:], esum[:], e2[:], op=ALU.add)
        erec = colp.tile([P, NCH], F32, tag="erec", bufs=1, name="erec")
        nc.vector.reciprocal(erec[:], esum[:])
        p1 = colp.tile([P, NCH], F32, tag="p1", bufs=1, name="p1")
        p2 = colp.tile([P, NCH], F32, tag="p2", bufs=1, name="p2")
        for src, dst in ((e1, p1), (e2, p2)):
            nc.vector.tensor_tensor(dst[:], src[:], erec[:], op=ALU.mult)
        brec = colp.tile([P, NCH], F32, tag="brec", bufs=1, name="brec")
        nc.vector.reciprocal(brec[:], beta_all[:])
        wg1m = colp.tile([P, NCH], F32, tag="wg1m", bufs=1, name="wg1m")
        nc.vector.tensor_scalar(wg1m[:], wg_c[:], -1.0, 1.0, op0=ALU.mult, op1=ALU.add)

        # ================= phase B: per-chunk scan =================
        def vtok_col(c):
            return vwin[:, :, FGUARD + c * CH: FGUARD + (c + 1) * CH]

        def norms(c):
            beta_c = beta_all[:, c:c + 1]
            res = {"beta": beta_c}
            # v: transpose + beta-scale -> vb_all[c]
            tpv = ps_t.tile([P, 2, CH], F16, tag="pst", name="tp_v")
            for pt in range(2):
                nc.tensor.transpose(tpv[:, pt, :],
                                    vwin[:, pt, FGUARD + c * CH: FGUARD + (c + 1) * CH],
                                    ident[:])
            nc.vector.tensor_scalar_mul(vb_all[:, c, :], tpv[:], beta_c)
            res["kTsl"] = [rollkq[:, pt, c, 0, :] for pt in range(2)]
            res["qTsl"] = [rollkq[:, pt, c, 1, :] for pt in range(2)]
            res["kqTsl"] = [rollkq[:, pt, c, :, :] for pt in range(2)]
            # token-major q/k for l2 stats
            toks = {}
            for t, koff in (("q", 1), ("k", 0)):
                tok = chk.tile([P, DV], F16, tag=f"{t}tok", bufs=2, name=f"{t}tok")
                tpt = ps_t.tile([P, 2, CH], F16, tag="pst", name=f"tp_{t}")
                for pt in range(2):
                    nc.tensor.transpose(tpt[:, pt, :], rollkq[:, pt, c, koff, :], ident[:])
                nc.scalar.copy(tok[:], tpt[:])
                toks[t] = tok
            ssqs = {}
            for t in ("q", "k"):
                scr = chk.tile([P, DV], F16, tag="sq_scr", bufs=2, name="sq_scr")
                ssq = chk.tile([P, 1], F32, tag=f"ssq{t}", bufs=2, name=f"ssq{t}")
                nc.vector.scalar_tensor_tensor(scr[:], toks[t][:], 1.0, toks[t][:],
                                               op0=ALU.mult, op1=ALU.mult, accum_out=ssq[:])
                ssqs[t] = ssq
            sroots = {}
            for t in ("q", "k"):
                sroot = chk.tile([P, 1], F32, tag=f"sroot{t}", bufs=2, name=f"sroot{t}")
                nc.scalar.activation(sroot[:], ssqs[t][:], AF.Sqrt, bias=eps_l2[:])
                sroots[t] = sroot
            for t in ("q", "k"):
                rinv = chk.tile([P, 1], F32, tag=f"rinv{t}", bufs=3, name=f"rinv{t}")
                nc.vector.reciprocal(rinv[:], sroots[t][:])
                res["rinv" + t] = rinv
            khat = chk.tile([P, DV], F16, tag="khat", bufs=2, name="khat")
            nc.vector.tensor_scalar_mul(khat[:], toks["k"][:], res["rinvk"][:])
            res["khat"] = khat
            khatb = chk.tile([P, DV], F16, tag="khatb", bufs=2, name="khatb")
            nc.vector.tensor_scalar_mul(khatb[:], khat[:], beta_c)
            res["khatb"] = khatb
            return res

        def mm_small(lhsT, rhs, name, evict_eng):
            ps = ps_med.tile([P, DV], F32, tag="psm", name=f"ps_{name}")
            nc.tensor.matmul(ps[:, :P], lhsT, rhs, start=True, stop=True)
            sb = chk.tile([P, P], F16, tag=name, bufs=1, name=name)
            if evict_eng == "v":
                nc.vector.tensor_copy(sb[:], ps[:, :P])
            else:
                nc.scalar.copy(sb[:], ps[:, :P])
            return sb

        def prepass(c, nr):
            beta_c = nr["beta"]
            rinvk, rinvq = nr["rinvk"], nr["rinvq"]
            # [Graw | Braw] = kraw @ [kraw | qraw]^T in one N=256 stream per pt
            gps = ps_med.tile([P, DV], F32, tag="psm", name="gps")
            for pt in range(2):
                nc.tensor.matmul(gps[:], nr["kTsl"][pt], nr["kqTsl"][pt],
                                 start=(pt == 0), stop=(pt == 1))
            # N1 = tril_strict * rowscale_{beta*rinvk}(Graw); Mt = N1^T rowscale rinvk
            br = chk.tile([P, 1], F32, tag="br", bufs=2, name="br")
            nc.vector.tensor_tensor(br[:], beta_c, rinvk[:], op=ALU.mult)
            N1 = chk.tile([P, P], F16, tag="N1", bufs=2, name="N1")
            nc.vector.scalar_tensor_tensor(N1[:], gps[:, :P], br[:], masklt[:],
                                           op0=ALU.mult, op1=ALU.mult)
            mps = ps_t.tile([P, 2, CH], F16, tag="pst", name="mps")
            nc.tensor.transpose(mps[:, 0, :], N1[:], ident[:])
            Mt = chk.tile([P, P], F16, tag="Mt", bufs=1, name="Mt")
            nc.vector.tensor_scalar_mul(Mt[:], mps[:, 0, :], rinvk[:])
            nps = ps_t.tile([P, 2, CH], F16, tag="pst", name="nps")
            nc.tensor.transpose(nps[:, 0, :], Mt[:], ident[:])
            Nt = chk.tile([P, P], F16, tag="Nt", bufs=1, name="Nt")
            nc.scalar.copy(Nt[:], nps[:, 0, :])
            # powers (Neumann truncated at A^15: products (I-M)(I+M2)(I+M4)(I+M8))
            N2 = mm_small(Mt[:], Nt[:], "N2", "v")
            M2 = mm_small(Nt[:], Mt[:], "M2", "s")
            N4 = mm_small(M2[:], N2[:], "N4", "v")
            M4 = mm_small(N2[:], M2[:], "M4", "s")
            N8 = mm_small(M4[:], N4[:], "N8", "v")
            # P chain: P1 = I - Mt ; P_{j+1} = P_j + Npow^T @ P_j
            P1 = chk.tile([P, P], F16, tag="P1", bufs=1, name="P1")
            nc.vector.tensor_tensor(P1[:], ident[:], Mt[:], op=ALU.subtract)
            Pc = P1
            for Npow, nm in ((N2, "P2"), (N4, "P3"), (N8, "TTt")):
                pps = ps_med.tile([P, DV], F32, tag="psm", name=f"pps_{nm}")
                nc.tensor.matmul(pps[:, :P], Npow[:], Pc[:], start=True, stop=True)
                nxt = chk.tile([P, P], F16, tag=nm, bufs=1 if nm != "TTt" else 2, name=nm)
                nc.vector.tensor_tensor(nxt[:], Pc[:], pps[:, :P], op=ALU.add)
                Pc = nxt
            TTt = Pc
            negTT = chk.tile([P, P], F16, tag="negTT", bufs=2, name="negTT")
            nc.vector.tensor_scalar_mul(negTT[:], TTt[:], -1.0)
            # attn^T = rowscale_{rinvk}(triu_incl * Braw)
            attnT = chk.tile([P, P], F16, tag="attnT", bufs=2, name="attnT")
            nc.vector.scalar_tensor_tensor(attnT[:], gps[:, P:], rinvk[:], maskut[:],
                                           op0=ALU.mult, op1=ALU.mult)
            # w^T(neg): [128, 2, 128]
            wTn = chk.tile([P, 2, CH], F16, tag="wTn", bufs=2, name="wTn")
            for kt in range(2):
                wps = ps_med.tile([P, DV], F32, tag="psm", name="wps")
                nc.tensor.matmul(wps[:, :P], nr["khatb"][:, kt * P:(kt + 1) * P], negTT[:],
                                 start=True, stop=True)
                nc.scalar.copy(wTn[:, kt, :], wps[:, :P])
            # u0 = T @ vb : lhsT = T^T
            ups = ps_med.tile([P, DV], F32, tag="psm", name="ups")
            nc.tensor.matmul(ups[:], TTt[:], vb_all[:, c, :], start=True, stop=True)
            u0 = chk.tile([P, DV], F16, tag="u0", bufs=2, name="u0")
            nc.scalar.copy(u0[:], ups[:])
            return {"TTt": TTt, "attnT": attnT, "wTn": wTn, "u0": u0,
                    "qTsl": nr["qTsl"], "khat": nr["khat"], "rinvq": rinvq}

        def serial(c, pr):
            if c == 0:
                u16 = pr["u0"]
            else:
                ups = ps_med.tile([P, DV], F32, tag="psm", name="ups_s")
                for kt in range(2):
                    nc.tensor.matmul(ups[:], pr["wTn"][:, kt, :], S16[:, kt, :],
                                     start=(kt == 0), stop=(kt == 1))
                u16 = chk.tile([P, DV], F16, tag="u16", bufs=2, name="u16")
                nc.vector.tensor_tensor(u16[:], ups[:], pr["u0"][:], op=ALU.add)
            # o
            ops = ps_med.tile([P, DV], F32, tag="psm", name="ops")
            if c == 0:
                nc.tensor.matmul(ops[:], pr["attnT"][:], u16[:], start=True, stop=True)
            else:
                for kt in range(2):
                    nc.tensor.matmul(ops[:], pr["qTsl"][kt], S16[:, kt, :],
                                     start=(kt == 0), stop=False)
                nc.tensor.matmul(ops[:], pr["attnT"][:], u16[:], start=False, stop=True)
            nc.vector.tensor_scalar_mul(o_all[:, c, :], ops[:], pr["rinvq"][:])
            # S += k^T u  (both kt halves in one PSUM bank, one fp16 add)
            dps = ps_big.tile([P, 2, DV], F32, tag="psb", name="dps")
            for kt in range(2):
                nc.tensor.matmul(dps[:, kt, :], pr["khat"][:, kt * P:(kt + 1) * P], u16[:],
                                 start=True, stop=True)
            if c == 0:
                nc.vector.tensor_copy(S16[:], dps[:])
            else:
                nc.vector.tensor_tensor(S16[:], S16[:], dps[:], op=ALU.add)

        # FIR long (PE diag matmuls) + short (DVE STT) for one n-tile
        def fir_tile(n):
            fch = {}
            for pt in range(2):
                ps = ps_big.tile([P, NTILE], F32, tag="psb", name="ps_ll")
                for j in range(FIR_L):
                    nc.tensor.matmul(ps[:], fdiag[:, pt, j, :],
                                     vwin[:, pt, n * NTILE + 1 + j: n * NTILE + 1 + j + NTILE],
                                     start=(j == 0), stop=(j == FIR_L - 1))
                ll = gat.tile([P, NTILE], F16, tag="llch", bufs=2, name="llch")
                nc.scalar.copy(ll[:], ps[:])
                fch[("ll", pt)] = ll
                ls = gat.tile([P, NTILE], F16, tag="lsch", bufs=2, name="lsch")
                bs = FGUARD - FIR_S + 1 + n * NTILE
                nc.vector.tensor_scalar_mul(ls[:], vwin[:, pt, bs:bs + NTILE], staps[:, pt, 0:1])
                for j in range(1, FIR_S):
                    sprod = gat.tile([P, NTILE], F16, tag="sprod", bufs=2, name="sprod")
                    nc.vector.tensor_scalar_mul(sprod[:], vwin[:, pt, bs + j:bs + j + NTILE],
                                                staps[:, pt, j:j + 1])
                    nc.vector.tensor_tensor(ls[:], ls[:], sprod[:], op=ALU.add)
                fch[("ls", pt)] = ls
            return fch

        # ================= phase C: gating + output projection =================
        def gate_out(lt, fch):
            off = (lt % CPN) * CH
            cs = lambda t: t[:, lt:lt + 1]
            toks = {}
            for f in ("ls", "ll"):
                tokt = gat.tile([P, DV], F16, tag=f"{f}tok", bufs=2, name=f"{f}tok")
                tp = ps_t.tile([P, 2, CH], F16, tag="pst", name=f"tp_{f}")
                for pt in range(2):
                    nc.tensor.transpose(tp[:, pt, :], fch[(f, pt)][:, off:off + CH], ident[:])
                nc.scalar.copy(tokt[:], tp[:])
                toks[f] = tokt
            # gating chain on the (otherwise idle) GPSIMD engine: only plain
            # tensor_tensor add/mult lower on Pool, so use broadcast scalars
            bc = lambda t: t[:, lt:lt + 1].to_broadcast((P, DV))
            mix = gat.tile([P, DV], F16, tag="gtmp", bufs=8, name="mix")
            nc.gpsimd.tensor_tensor(mix[:], vb_all[:, lt, :], bc(brec), op=ALU.mult)
            t1 = gat.tile([P, DV], F16, tag="gtmp", bufs=8, name="t1g")
            nc.gpsimd.tensor_tensor(t1[:], toks["ls"][:], bc(p1), op=ALU.mult)
            mix2 = gat.tile([P, DV], F16, tag="gtmp", bufs=8, name="mix2")
            nc.gpsimd.tensor_tensor(mix2[:], t1[:], mix[:], op=ALU.add)
            t2 = gat.tile([P, DV], F16, tag="gtmp", bufs=8, name="t2g")
            nc.gpsimd.tensor_tensor(t2[:], toks["ll"][:], bc(p2), op=ALU.mult)
            mix3 = gat.tile([P, DV], F16, tag="gtmp", bufs=8, name="mix3")
            nc.gpsimd.tensor_tensor(mix3[:], t2[:], mix2[:], op=ALU.add)
            tmp = gat.tile([P, DV], F16, tag="gtmp", bufs=8, name="tmpg")
            nc.gpsimd.tensor_tensor(tmp[:], mix3[:], bc(wg1m), op=ALU.mult)
            t3 = gat.tile([P, DV], F16, tag="gtmp", bufs=8, name="t3g")
            nc.gpsimd.tensor_tensor(t3[:], o_all[:, lt, :], bc(wg_c), op=ALU.mult)
            om = gat.tile([P, DV], F16, tag="gtmp", bufs=8, name="om")
            nc.gpsimd.tensor_tensor(om[:], t3[:], tmp[:], op=ALU.add)
            scr = gat.tile([P, DV], F16, tag="scr_g", bufs=2, name="scr_g")
            ssq = gat.tile([P, 1], F32, tag="ssq_g", bufs=2, name="ssq_g")
            nc.vector.scalar_tensor_tensor(scr[:], om[:], 1.0, om[:],
                                           op0=ALU.mult, op1=ALU.mult, accum_out=ssq[:])
            srt = gat.tile([P, 1], F32, tag="srt_g", bufs=2, name="srt_g")
            nc.scalar.activation(srt[:], ssq[:], AF.Sqrt, bias=eps_rms[:], scale=1.0 / DV)
            rin = gat.tile([P, 1], F32, tag="rin_g", bufs=2, name="rin_g")
            nc.vector.reciprocal(rin[:], srt[:])
            # output projection: transpose unnormalized om; fold the RMS scale
            # into the PSUM eviction (per-partition scale on out rows = tokens)
            onT = gat.tile([P, 2, CH], F16, tag="onT", bufs=2, name="onT")
            tp = ps_t.tile([P, 2, CH], F16, tag="pst", name="tp_on")
            for pt in range(2):
                nc.tensor.transpose(tp[:, pt, :], om[:, pt * P:(pt + 1) * P], ident[:])
            nc.scalar.copy(onT[:], tp[:])
            out_sb = gat.tile([P, D], F16, tag="out_sb", bufs=2, name="out_sb")
            for nt2 in range(2):
                ops = ps_big.tile([P, NTILE], F32, tag="psb", name="ops_o")
                for kt in range(2):
                    nc.tensor.matmul(ops[:], onT[:, kt, :],
                                     wo[:, kt, nt2 * NTILE:(nt2 + 1) * NTILE],
                                     start=(kt == 0), stop=(kt == 1))
                nc.scalar.activation(out_sb[:, nt2 * NTILE:(nt2 + 1) * NTILE], ops[:],
                                     AF.Identity, scale=rin[:])
            nc.sync.dma_start(out_d[lt * CH:(lt + 1) * CH, :], out_sb[:])

        # ---- emit phases B & C interleaved (C one tile behind) ----
        pending = None       # (c, prepass result)
        pend_tile = None     # (lts, fch)
        for n in range(NT):
            fch = fir_tile(n)
            for c in range(n * CPN, (n + 1) * CPN):
                nr = norms(c)
                pr = prepass(c, nr)
                if pending is not None:
                    serial(pending[0], pending[1])
                pending = (c, pr)
            if pend_tile is not None:
                for lt in pend_tile[0]:
                    gate_out(lt, pend_tile[1])
            pend_tile = (list(range(n * CPN, (n + 1) * CPN)), fch)
        serial(pending[0], pending[1])
        for lt in pend_tile[0]:
            gate_out(lt, pend_tile[1])

    nc.compile()
    return nc


# ---------------- host side ----------------

def _diag_stack(taps):
    """taps [C, K] -> [2, K, 128, 128] diag matrices."""
    C, K = taps.shape
    out = np.zeros((2, K, P, P), np.float32)
    for pt in range(2):
        for j in range(K):
            np.fill_diagonal(out[pt, j], taps[pt * P:(pt + 1) * P, j])
    return out


def make_core_inputs(inputs, b, h, L):
    f = lambda a: np.ascontiguousarray(np.asarray(a, np.float32))
    x = f(inputs['hidden_states'])[b]          # [L, D]
    temp = float(np.exp(np.asarray(inputs['log_temp'], np.float64)[h]))
    wsm = np.concatenate([
        f(inputs['Wb'])[:, h:h + 1],
        f(inputs['Wg'])[:, h:h + 1],
        f(inputs['Wl'])[:, 3 * h:3 * h + 3] / temp], axis=1)
    bias5 = np.array([0.0, float(np.asarray(inputs['bg'], np.float64)[h]),
                      *(np.asarray(inputs['bl'], np.float64)[3 * h:3 * h + 3] / temp)],
                     np.float32)
    ct = np.stack([
        f(inputs['conv_q'])[h * DK:(h + 1) * DK].reshape(2, P, CONV_K),
        f(inputs['conv_k'])[h * DK:(h + 1) * DK].reshape(2, P, CONV_K),
        f(inputs['conv_v'])[h * DV:(h + 1) * DV].reshape(2, P, CONV_K)])  # [3, 2, 128, 4]
    # residual FIR taps: fir = delta + r  ->  local = v + FIR_r(v); softmax sums to 1
    fs = f(inputs['fir_short'])[h].copy()   # [DV, 3]
    fs[:, -1] -= 1.0
    fl = f(inputs['fir_long'])[h].copy()    # [DV, 31]
    fl[:, -1] -= 1.0
    fd = _diag_stack(fl).astype(np.float16)
    st = fs.reshape(2, P, FIR_S)
    wo = f(inputs['rms_w'])[:, None] * f(inputs['Wo'])[h * DV:(h + 1) * DV]
    h16 = np.float16
    return dict(
        xT=np.ascontiguousarray(x.T).astype(h16),
        wq=np.ascontiguousarray(f(inputs['Wq'])[:, h * DK:(h + 1) * DK]).astype(h16),
        wk=np.ascontiguousarray(f(inputs['Wk'])[:, h * DK:(h + 1) * DK]).astype(h16),
        wv=np.ascontiguousarray(f(inputs['Wv'])[:, h * DV:(h + 1) * DV]).astype(h16),
        wsm=wsm.astype(h16), bias5=bias5,
        ctaps=ct, fdiag=fd, staps=st.astype(np.float32), wo=wo.astype(h16),
        ident=np.eye(P, dtype=h16),
        masklt=np.tril(np.ones((P, P), h16), -1),
        maskut=np.triu(np.ones((P, P), h16), 0),
    )


_NC_CACHE = {}


def _get_nc(L):
    if L not in _NC_CACHE:
        _NC_CACHE[L] = build(L)
    return _NC_CACHE[L]


def kernel(**inputs):
    x = np.asarray(inputs['hidden_states'])
    Bx, L, _ = x.shape
    nc = _get_nc(L)
    in_maps = [make_core_inputs(inputs, c // H, c % H, L) for c in range(8)]
    res = run_bass_kernel_spmd(nc, in_maps, core_ids=list(range(8)))
    out = np.zeros((Bx, L, D), np.float32)
    for c in range(8):
        out[c // H] += res.results[c]['out'].astype(np.float32)
    return out


# revision 22
# speedup vs baseline: 1.9161x; 1.1998x over previous
"""DeltaNet Bass kernel for Trainium2, 8-core SPMD — fp16 matmul pipeline.

Sharding: core = (b, h) for b in 0..1, h in 0..3  (b*4 + h).
Each core computes the full per-(batch,head) pipeline and its partial
output projection out_partial[L, D] in fp16; the host sums the 4
head-partials per batch.

Device pipeline per core (PSUM accumulate fp32, operands fp16):
  phase A (per 512-col tile, all 8 tiles first): q/k/v/small projections
          (PE, fp16), causal 4-tap conv (DVE STT chain, fp16 4x mode),
          SiLU evict (ACT) into persistent SBUF: rollkq (chan-major k|q
          interleaved per 128-token chunk) and vwin (guarded chan-major v).
  phase B prologue: beta + gate columns from rows scratch (one Sigmoid /
          Exp table load each).
  phase B (per 128-token chunk): PE transposes (fp16 PSUM), l2norm stats,
          UT-transform T^T via Neumann product, chunk-local prepass,
          serial scan (u = u0 - w S, o = q S + attn u, S += k^T u) with
          fp32 S master + fp16 S copy; FIR-long diag matmuls + FIR-short
          STT chains interleaved per tile to fill PE gaps.
  phase C (per chunk, pipelined one tile behind): FIR transposes,
          hierarchical gating (DVE fp16), RMSNorm, output projection.
"""
import numpy as np
import ml_dtypes
from contextlib import ExitStack

import concourse.bass as bass
import concourse.tile as tile
from concourse import bacc, mybir
from concourse.bass_utils import run_bass_kernel_spmd

F32 = mybir.dt.float32
F16 = mybir.dt.float16
AF = mybir.ActivationFunctionType
ALU = mybir.AluOpType

B, D, H, DK, DV = 2, 1024, 4, 256, 256
CONV_K, FIR_S, FIR_L = 4, 3, 31
CH = 128          # scan chunk (token tile)
NTILE = 512       # column tile for projections / FIR
P = 128
KT = D // P       # 8 contraction tiles over D
GUARD = CONV_K - 1
EPS_RMS = 1e-5
FGUARD = FIR_L    # guard cols ahead of token 0 in vwin


def build(L=4096):
    NT = L // NTILE
    NCH = L // CH
    CPN = NTILE // CH  # chunks per n-tile (4)

    nc = bacc.Bacc("TRN2", target_bir_lowering=False, debug=False, num_devices=8)

    xT_d = nc.dram_tensor("xT", [D, L], F16, kind="ExternalInput").ap()
    wq_d = nc.dram_tensor("wq", [D, DK], F16, kind="ExternalInput").ap()
    wk_d = nc.dram_tensor("wk", [D, DK], F16, kind="ExternalInput").ap()
    wv_d = nc.dram_tensor("wv", [D, DV], F16, kind="ExternalInput").ap()
    wsm_d = nc.dram_tensor("wsm", [D, 5], F16, kind="ExternalInput").ap()
    bias5_d = nc.dram_tensor("bias5", [5], F32, kind="ExternalInput").ap()
    # conv taps per (tensor, pt): [3, 2, 128, 4]
    ctaps_d = nc.dram_tensor("ctaps", [3, 2, P, CONV_K], F32, kind="ExternalInput").ap()
    # fir long-residual diagonal matrices: [pt=2, 31, 128, 128]
    fdiag_d = nc.dram_tensor("fdiag", [2, FIR_L, P, P], F16, kind="ExternalInput").ap()
    # fir short-residual taps: [2, 128, 3]
    staps_d = nc.dram_tensor("staps", [2, P, FIR_S], F32, kind="ExternalInput").ap()
    wo_d = nc.dram_tensor("wo", [DV, D], F16, kind="ExternalInput").ap()
    ident_d = nc.dram_tensor("ident", [P, P], F16, kind="ExternalInput").ap()
    masklt_d = nc.dram_tensor("masklt", [P, P], F16, kind="ExternalInput").ap()  # strict lower
    maskut_d = nc.dram_tensor("maskut", [P, P], F16, kind="ExternalInput").ap()  # upper incl diag
    out_d = nc.dram_tensor("out", [L, D], F16, kind="ExternalOutput").ap()

    with tile.TileContext(nc) as tc, ExitStack() as ctx:
        # ---------------- pools ----------------
        const = ctx.enter_context(tc.tile_pool(name="const", bufs=1))
        bigw = ctx.enter_context(tc.tile_pool(name="bigw", bufs=1))
        persist = ctx.enter_context(tc.tile_pool(name="persist", bufs=1))
        xtp = ctx.enter_context(tc.tile_pool(name="xtp", bufs=2))
        prep = ctx.enter_context(tc.tile_pool(name="prep", bufs=1))
        colp = ctx.enter_context(tc.tile_pool(name="colp", bufs=1))
        chk = ctx.enter_context(tc.tile_pool(name="chk", bufs=1))
        sp = ctx.enter_context(tc.tile_pool(name="sp", bufs=1))
        gat = ctx.enter_context(tc.tile_pool(name="gat", bufs=1))
        dram = ctx.enter_context(tc.tile_pool(name="dram", bufs=1, space="DRAM"))
        ps_big = ctx.enter_context(tc.tile_pool(name="ps_big", bufs=2, space="PSUM"))
        ps_med = ctx.enter_context(tc.tile_pool(name="ps_med", bufs=3, space="PSUM"))
        ps_t = ctx.enter_context(tc.tile_pool(name="ps_t", bufs=2, space="PSUM"))
        ps_fir = ctx.enter_context(tc.tile_pool(name="ps_fir", bufs=1, space="PSUM"))

        # ---------------- DRAM scratch ----------------
        rows_d = dram.tile([5, L], F32, name="rows_sc")

        # ---------------- constants / weights ----------------
        wq = bigw.tile([P, KT, DK], F16, tag="wq")
        nc.sync.dma_start(wq[:], wq_d.rearrange("(kt p) m -> p kt m", p=P))
        wk = bigw.tile([P, KT, DK], F16, tag="wk")
        nc.sync.dma_start(wk[:], wk_d.rearrange("(kt p) m -> p kt m", p=P))
        wv = bigw.tile([P, KT, DV], F16, tag="wv")
        nc.sync.dma_start(wv[:], wv_d.rearrange("(kt p) m -> p kt m", p=P))
        wsm = const.tile([P, KT, 5], F16)
        nc.sync.dma_start(wsm[:], wsm_d.rearrange("(kt p) m -> p kt m", p=P))
        ident = const.tile([P, P], F16)
        nc.sync.dma_start(ident[:], ident_d)
        masklt = const.tile([P, P], F16)
        nc.sync.dma_start(masklt[:], masklt_d)
        maskut = const.tile([P, P], F16)
        nc.sync.dma_start(maskut[:], maskut_d)
        bias5 = const.tile([5, 1], F32)
        nc.sync.dma_start(bias5[:], bias5_d.rearrange("(m o) -> m o", o=1))
        ctaps = const.tile([P, 3, 2, CONV_K], F32, name="ctaps")
        nc.sync.dma_start(ctaps[:], ctaps_d.rearrange("t pt p j -> p t pt j"))
        staps = const.tile([P, 2, FIR_S], F32, name="staps")
        nc.sync.dma_start(staps[:], staps_d.rearrange("pt p j -> p pt j"))
        fdiag = bigw.tile([P, 2, FIR_L, P], F16, tag="fdiag", name="fdiag")
        nc.sync.dma_start(fdiag[:], fdiag_d.rearrange("pt j p q -> p pt j q"))
        wo = bigw.tile([P, 2, D], F16, tag="wo", name="wo")
        nc.sync.dma_start(wo[:], wo_d.rearrange("(kt p) m -> p kt m", p=P))
        zeros3 = const.tile([P, GUARD], F16)
        nc.vector.memset(zeros3[:], 0.0)
        eps_l2 = const.tile([P, 1], F32)
        nc.vector.memset(eps_l2[:], 1e-6)
        eps_rms = const.tile([P, 1], F32)
        nc.vector.memset(eps_rms[:], EPS_RMS)

        # ---------------- persistent SBUF state ----------------
        # chan-major post-silu k|q interleaved per chunk: [P, pt, chunk, (k|q), CH]
        rollkq = persist.tile([P, 2, NCH, 2, CH], F16, name="rollkq")
        # chan-major post-silu v with FGUARD leading guard cols
        vwin = persist.tile([P, 2, FGUARD + L], F16, name="vwin")
        nc.vector.memset(vwin[:, :, 0:FGUARD], 0.0)
        # per-chunk outputs of the scan + beta-scaled v (token-major)
        o_all = persist.tile([P, NCH, DV], F16, name="o_all")
        vb_all = persist.tile([P, NCH, DV], F16, name="vb_all")

        S16 = sp.tile([P, 2, DV], F16, name="S16")

        TENS = ("q", "k", "v")
        W_OF = {"q": wq, "k": wk, "v": wv}
        prev_pre = {}

        # ================= phase A: projections + conv + silu =================
        def phaseA(n):
            xt = xtp.tile([P, KT, NTILE], F16, tag="xt", name="xt")
            nc.sync.dma_start(xt[:], xT_d[:, n * NTILE:(n + 1) * NTILE]
                              .rearrange("(kt p) t -> p kt t", p=P))
            for t in TENS:
                for pt in range(2):
                    ps = ps_big.tile([P, NTILE], F32, tag="psb", name=f"ps_{t}{pt}")
                    for kt in range(KT):
                        nc.tensor.matmul(ps[:], W_OF[t][:, kt, pt * P:(pt + 1) * P],
                                         xt[:, kt, :], start=(kt == 0), stop=(kt == KT - 1))
                    key = f"pre{t}{pt}"
                    pre = prep.tile([P, GUARD + NTILE], F16, tag=key, bufs=2, name=key)
                    if n == 0:
                        nc.scalar.copy(pre[:, 0:GUARD], zeros3[:])
                    else:
                        nc.scalar.copy(pre[:, 0:GUARD], prev_pre[key][:, NTILE:NTILE + GUARD])
                    nc.scalar.copy(pre[:, GUARD:], ps[:])
                    prev_pre[key] = pre
                    # conv: 4-tap chain. q/k on DVE in TS+TT form (tensor_scalar
                    # and tensor_tensor hit the 2x 16-bit DVE mode; STT does
                    # not). v on the idle Pool engine (TT-only, broadcast taps).
                    ti = TENS.index(t)
                    acc = prep.tile([P, NTILE], F16, tag="cacc", bufs=2, name="cacc")
                    if t == "v":
                        bct = lambda j: ctaps[:, ti, pt, j:j + 1].to_broadcast((P, NTILE))
                        nc.gpsimd.tensor_tensor(acc[:], pre[:, 0:NTILE], bct(0), op=ALU.mult)
                        for j in range(1, CONV_K):
                            prod = prep.tile([P, NTILE], F16, tag="cprod", bufs=2, name="cprod")
                            nc.gpsimd.tensor_tensor(prod[:], pre[:, j:j + NTILE], bct(j),
                                                    op=ALU.mult)
                            nc.gpsimd.tensor_tensor(acc[:], acc[:], prod[:], op=ALU.add)
                    else:
                        nc.vector.tensor_scalar_mul(acc[:], pre[:, 0:NTILE],
                                                    ctaps[:, ti, pt, 0:1])
                        for j in range(1, CONV_K):
                            prod = prep.tile([P, NTILE], F16, tag="cprod", bufs=2, name="cprod")
                            nc.vector.tensor_scalar_mul(prod[:], pre[:, j:j + NTILE],
                                                        ctaps[:, ti, pt, j:j + 1])
                            nc.vector.tensor_tensor(acc[:], acc[:], prod[:], op=ALU.add)
                    # silu -> persistent layout
                    if t == "v":
                        nc.scalar.activation(
                            vwin[:, pt, FGUARD + n * NTILE: FGUARD + (n + 1) * NTILE],
                            acc[:], AF.Silu)
                    else:
                        koff = 0 if t == "k" else 1
                        nc.scalar.activation(
                            rollkq[:, pt, n * CPN:(n + 1) * CPN, koff, :],
                            acc[:], AF.Silu)
            # small projections [5, NTILE]
            ps5 = ps_big.tile([P, NTILE], F32, tag="psb", name="ps5")
            for kt in range(KT):
                nc.tensor.matmul(ps5[:5, :], wsm[:, kt, :], xt[:, kt, :],
                                 start=(kt == 0), stop=(kt == KT - 1))
            rows_sb = colp.tile([5, NTILE], F32, tag="rows_sb", bufs=2, name="rows_sb")
            nc.scalar.activation(rows_sb[:], ps5[:5, :], AF.Identity, bias=bias5[:])
            nc.sync.dma_start(rows_d[:, n * NTILE:(n + 1) * NTILE], rows_sb[:])

        for n in range(NT):
            phaseA(n)

        # ================= phase B prologue: beta + gate columns =================
        def col_from_row(r, name):
            t = colp.tile([P, NCH], F32, tag=name, bufs=1, name=name)
            nc.sync.dma_start(t[:], rows_d[r].rearrange("(nn p) -> p nn", p=P))
            return t

        beta_all = colp.tile([P, NCH], F32, tag="beta_all", bufs=1, name="beta_all")
        braw = col_from_row(0, "braw")
        nc.scalar.activation(beta_all[:], braw[:], AF.Sigmoid)
        wg_c = colp.tile([P, NCH], F32, tag="wg_c", bufs=1, name="wg_c")
        wgraw = col_from_row(1, "wgraw")
        nc.scalar.activation(wg_c[:], wgraw[:], AF.Sigmoid)
        l0_c = col_from_row(2, "l0_c")
        l1_c = col_from_row(3, "l1_c")
        l2_c = col_from_row(4, "l2_c")
        mx = colp.tile([P, NCH], F32, tag="mx", bufs=1, name="mx")
        nc.vector.tensor_tensor(mx[:], l0_c[:], l1_c[:], op=ALU.max)
        nc.vector.tensor_tensor(mx[:], mx[:], l2_c[:], op=ALU.max)
        e0 = colp.tile([P, NCH], F32, tag="e0", bufs=1, name="e0")
        e1 = colp.tile([P, NCH], F32, tag="e1", bufs=1, name="e1")
        e2 = colp.tile([P, NCH], F32, tag="e2", bufs=1, name="e2")
        for src, dst in ((l0_c, e0), (l1_c, e1), (l2_c, e2)):
            nc.vector.tensor_tensor(dst[:], src[:], mx[:], op=ALU.subtract)
            nc.scalar.activation(dst[:], dst[:], AF.Exp)
        esum = colp.tile([P, NCH], F32, tag="esum", bufs=1, name="esum")
        nc.vector.tensor_tensor(esum[:], e0[:], e1[:], op=ALU.add)
        nc.vector.tensor_tensor(esum[:], esum[:], e2[:], op=ALU.add)
        erec = colp.tile([P, NCH], F32, tag="erec", bufs=1, name="erec")
        nc.vector.reciprocal(erec[:], esum[:])
        p1 = colp.tile([P, NCH], F32, tag="p1", bufs=1, name="p1")
        p2 = colp.tile([P, NCH], F32, tag="p2", bufs=1, name="p2")
        for src, dst in ((e1, p1), (e2, p2)):
            nc.vector.tensor_tensor(dst[:], src[:], erec[:], op=ALU.mult)
        brec = colp.tile([P, NCH], F32, tag="brec", bufs=1, name="brec")
        nc.vector.reciprocal(brec[:], beta_all[:])
        wg1m = colp.tile([P, NCH], F32, tag="wg1m", bufs=1, name="wg1m")
        nc.vector.tensor_scalar(wg1m[:], wg_c[:], -1.0, 1.0, op0=ALU.mult, op1=ALU.add)

        # ================= phase B: per-chunk scan =================
        def vtok_col(c):
            return vwin[:, :, FGUARD + c * CH: FGUARD + (c + 1) * CH]

        # ---- software-pipelined chunk stages ----
        # S1(c): token-major stats + G  (emitted 3 iterations ahead)
        # S2(c): scalings N1/Mt/Nt/attnT/khat/khatb  (2 ahead)
        # S3(c): Neumann ladder + P-chain + wTn/u0  (1 ahead)
        # S4(c): serial scan step  (current)
        # FIR-long matmuls are dripped between dependent S3 steps so the
        # in-order PE queue always has ready work.
        st1, st2, st3 = {}, {}, {}

        def S1(c):
            beta_c = beta_all[:, c:c + 1]
            res = {"beta": beta_c}
            tpv = ps_t.tile([P, 2, CH], F16, tag="pst", name="tp_v")
            for pt in range(2):
                nc.tensor.transpose(tpv[:, pt, :],
                                    vwin[:, pt, FGUARD + c * CH: FGUARD + (c + 1) * CH],
                                    ident[:])
            nc.vector.tensor_scalar_mul(vb_all[:, c, :], tpv[:], beta_c)
            res["kTsl"] = [rollkq[:, pt, c, 0, :] for pt in range(2)]
            res["qTsl"] = [rollkq[:, pt, c, 1, :] for pt in range(2)]
            res["kqTsl"] = [rollkq[:, pt, c, :, :] for pt in range(2)]
            toks = {}
            for t, koff in (("q", 1), ("k", 0)):
                tok = chk.tile([P, DV], F16, tag=f"{t}tok", bufs=3, name=f"{t}tok")
                tpt = ps_t.tile([P, 2, CH], F16, tag="pst", name=f"tp_{t}")
                for pt in range(2):
                    nc.tensor.transpose(tpt[:, pt, :], rollkq[:, pt, c, koff, :], ident[:])
                nc.scalar.copy(tok[:], tpt[:])
                toks[t] = tok
            res["ktok"] = toks["k"]
            # [Graw | Braw] = kraw @ [kraw | qraw]^T ; evict fp16 to free the bank
            gps = ps_med.tile([P, DV], F32, tag="psm", name="gps")
            for pt in range(2):
                nc.tensor.matmul(gps[:], res["kTsl"][pt], res["kqTsl"][pt],
                                 start=(pt == 0), stop=(pt == 1))
            g16 = chk.tile([P, DV], F16, tag="g16", bufs=3, name="g16")
            nc.scalar.copy(g16[:], gps[:])
            res["g16"] = g16
            for t in ("q", "k"):
                scr = chk.tile([P, DV], F16, tag="sq_scr", bufs=2, name="sq_scr")
                ssq = chk.tile([P, 1], F32, tag=f"ssq{t}", bufs=2, name=f"ssq{t}")
                nc.vector.scalar_tensor_tensor(scr[:], toks[t][:], 1.0, toks[t][:],
                                               op0=ALU.mult, op1=ALU.mult, accum_out=ssq[:])
                sroot = chk.tile([P, 1], F32, tag=f"sroot{t}", bufs=2, name=f"sroot{t}")
                nc.scalar.activation(sroot[:], ssq[:], AF.Sqrt, bias=eps_l2[:])
                rinv = chk.tile([P, 1], F32, tag=f"rinv{t}", bufs=5, name=f"rinv{t}")
                nc.vector.reciprocal(rinv[:], sroot[:])
                res["rinv" + t] = rinv
            st1[c] = res

        def S2(c):
            res = st1.pop(c)
            beta_c, rinvk = res["beta"], res["rinvk"]
            khat = chk.tile([P, DV], F16, tag="khat", bufs=4, name="khat")
            nc.vector.tensor_scalar_mul(khat[:], res["ktok"][:], rinvk[:])
            res["khat"] = khat
            khatb = chk.tile([P, DV], F16, tag="khatb", bufs=3, name="khatb")
            nc.vector.tensor_scalar_mul(khatb[:], khat[:], beta_c)
            res["khatb"] = khatb
            br = chk.tile([P, 1], F32, tag="br", bufs=2, name="br")
            nc.vector.tensor_tensor(br[:], beta_c, rinvk[:], op=ALU.mult)
            N1 = chk.tile([P, P], F16, tag="N1", bufs=2, name="N1")
            nc.vector.scalar_tensor_tensor(N1[:], res["g16"][:, :P], br[:], masklt[:],
                                           op0=ALU.mult, op1=ALU.mult)
            mps = ps_t.tile([P, 2, CH], F16, tag="pst", name="mps")
            nc.tensor.transpose(mps[:, 0, :], N1[:], ident[:])
            Mt = chk.tile([P, P], F16, tag="Mt", bufs=3, name="Mt")
            nc.vector.tensor_scalar_mul(Mt[:], mps[:, 0, :], rinvk[:])
            nc.tensor.transpose(mps[:, 1, :], Mt[:], ident[:])
            Nt = chk.tile([P, P], F16, tag="Nt", bufs=3, name="Nt")
            nc.scalar.copy(Nt[:], mps[:, 1, :])
            res["Mt"], res["Nt"] = Mt, Nt
            attnT = chk.tile([P, P], F16, tag="attnT", bufs=4, name="attnT")
            nc.vector.scalar_tensor_tensor(attnT[:], res["g16"][:, P:], rinvk[:], maskut[:],
                                           op0=ALU.mult, op1=ALU.mult)
            res["attnT"] = attnT
            st2[c] = res

        def mm_small(lhsT, rhs, name, evict_eng):
            ps = ps_med.tile([P, DV], F32, tag="psm", name=f"ps_{name}")
            nc.tensor.matmul(ps[:, :P], lhsT, rhs, start=True, stop=True)
            sb = chk.tile([P, P], F16, tag=name, bufs=2, name=name)
            if evict_eng == "v":
                nc.vector.tensor_copy(sb[:], ps[:, :P])
            else:
                nc.scalar.copy(sb[:], ps[:, :P])
            return sb

        def S3(c, drip):
            res = st2.pop(c)
            Mt, Nt = res["Mt"], res["Nt"]
            # Neumann truncated at A^15: T^T = (I-Mt)(I+Mt^2)(I+Mt^4)(I+Mt^8)
            P1 = chk.tile([P, P], F16, tag="P1", bufs=2, name="P1")
            nc.vector.tensor_tensor(P1[:], ident[:], Mt[:], op=ALU.subtract)
            N2 = mm_small(Mt[:], Nt[:], "N2", "v")
            M2 = mm_small(Nt[:], Mt[:], "M2", "s")
            drip(3)
            N4 = mm_small(M2[:], N2[:], "N4", "v")
            M4 = mm_small(N2[:], M2[:], "M4", "s")
            drip(3)
            N8 = mm_small(M4[:], N4[:], "N8", "v")
            drip(2)
            Pc = P1
            for Npow, nm in ((N2, "P2"), (N4, "P3"), (N8, "TTt")):
                pps = ps_med.tile([P, DV], F32, tag="psm", name=f"pps_{nm}")
                nc.tensor.matmul(pps[:, :P], Npow[:], Pc[:], start=True, stop=True)
                nxt = chk.tile([P, P], F16, tag=nm, bufs=2, name=nm)
                nc.vector.tensor_tensor(nxt[:], Pc[:], pps[:, :P], op=ALU.add)
                Pc = nxt
                drip(2)
            TTt = Pc
            negTT = chk.tile([P, P], F16, tag="negTT", bufs=2, name="negTT")
            nc.vector.tensor_scalar_mul(negTT[:], TTt[:], -1.0)
            drip(2)
            wTn = chk.tile([P, 2, CH], F16, tag="wTn", bufs=3, name="wTn")
            for kt in range(2):
                wps = ps_med.tile([P, DV], F32, tag="psm", name="wps")
                nc.tensor.matmul(wps[:, :P], res["khatb"][:, kt * P:(kt + 1) * P], negTT[:],
                                 start=True, stop=True)
                nc.scalar.copy(wTn[:, kt, :], wps[:, :P])
            res["wTn"] = wTn
            ups = ps_med.tile([P, DV], F32, tag="psm", name="ups")
            nc.tensor.matmul(ups[:], TTt[:], vb_all[:, c, :], start=True, stop=True)
            u0 = chk.tile([P, DV], F16, tag="u0", bufs=3, name="u0")
            nc.scalar.copy(u0[:], ups[:])
            res["u0"] = u0
            st3[c] = res

        def S4(c):
            pr = st3.pop(c)
            if c == 0:
                u16 = pr["u0"]
            else:
                ups = ps_med.tile([P, DV], F32, tag="psm", name="ups_s")
                for kt in range(2):
                    nc.tensor.matmul(ups[:], pr["wTn"][:, kt, :], S16[:, kt, :],
                                     start=(kt == 0), stop=(kt == 1))
                u16 = chk.tile([P, DV], F16, tag="u16", bufs=2, name="u16")
                nc.vector.tensor_tensor(u16[:], ups[:], pr["u0"][:], op=ALU.add)
            ops = ps_med.tile([P, DV], F32, tag="psm", name="ops")
            if c == 0:
                nc.tensor.matmul(ops[:], pr["attnT"][:], u16[:], start=True, stop=True)
            else:
                for kt in range(2):
                    nc.tensor.matmul(ops[:], pr["qTsl"][kt], S16[:, kt, :],
                                     start=(kt == 0), stop=False)
                nc.tensor.matmul(ops[:], pr["attnT"][:], u16[:], start=False, stop=True)
            nc.vector.tensor_scalar_mul(o_all[:, c, :], ops[:], pr["rinvq"][:])
            # S += k^T u  (both kt halves in one PSUM bank, one fp16 add)
            dps = ps_big.tile([P, 2, DV], F32, tag="psb", name="dps")
            for kt in range(2):
                nc.tensor.matmul(dps[:, kt, :], pr["khat"][:, kt * P:(kt + 1) * P], u16[:],
                                 start=True, stop=True)
            if c == 0:
                nc.vector.tensor_copy(S16[:], dps[:])
            else:
                nc.vector.tensor_tensor(S16[:], S16[:], dps[:], op=ALU.add)

        # ---- FIR long: per-iteration batches of PE matmuls, dripped via S3 ----
        fir_state = {}
        fch_of = {}   # n -> fch dict

        def fir_mm_thunks(k):
            """Iteration k (0..NCH-1): list of closures, one FIR matmul each."""
            n, b = k // CPN, k % CPN
            pt, half = b // 2, b % 2
            st = fir_state.setdefault(n, {})
            thunks = []
            if half == 0:
                ps = ps_fir.tile([P, NTILE], F32, tag="fir", name="ps_ll")
                st[pt] = ps
                taps = range(0, 16)
            else:
                ps = st[pt]
                taps = range(16, FIR_L)
            for j in taps:
                def mk(j=j, ps=ps, pt=pt, n=n):
                    nc.tensor.matmul(ps[:], fdiag[:, pt, j, :],
                                     vwin[:, pt, n * NTILE + 1 + j: n * NTILE + 1 + j + NTILE],
                                     start=(j == 0), stop=(j == FIR_L - 1))
                thunks.append(mk)
            return thunks

        def fir_finish(k):
            """Evicts + FIR-short chain for iteration k's batch."""
            n, b = k // CPN, k % CPN
            pt, half = b // 2, b % 2
            fch = fch_of.setdefault(n, {})
            if half == 1:
                ll = gat.tile([P, NTILE], F16, tag="llch", bufs=6, name="llch")
                nc.scalar.copy(ll[:], fir_state[n][pt][:])
                fch[("ll", pt)] = ll
            else:
                ls = gat.tile([P, NTILE], F16, tag="lsch", bufs=6, name="lsch")
                bs = FGUARD - FIR_S + 1 + n * NTILE
                nc.vector.tensor_scalar_mul(ls[:], vwin[:, pt, bs:bs + NTILE],
                                            staps[:, pt, 0:1])
                for j in range(1, FIR_S):
                    sprod = gat.tile([P, NTILE], F16, tag="sprod", bufs=2, name="sprod")
                    nc.vector.tensor_scalar_mul(sprod[:], vwin[:, pt, bs + j:bs + j + NTILE],
                                                staps[:, pt, j:j + 1])
                    nc.vector.tensor_tensor(ls[:], ls[:], sprod[:], op=ALU.add)
                fch[("ls", pt)] = ls

        # ================= phase C: gating + output projection =================
        def gate_out(lt, fch):
            off = (lt % CPN) * CH
            cs = lambda t: t[:, lt:lt + 1]
            toks = {}
            for f in ("ls", "ll"):
                tokt = gat.tile([P, DV], F16, tag=f"{f}tok", bufs=2, name=f"{f}tok")
                tp = ps_t.tile([P, 2, CH], F16, tag="pst", name=f"tp_{f}")
                for pt in range(2):
                    nc.tensor.transpose(tp[:, pt, :], fch[(f, pt)][:, off:off + CH], ident[:])
                nc.scalar.copy(tokt[:], tp[:])
                toks[f] = tokt
            # gating chain on the (otherwise idle) GPSIMD engine: only plain
            # tensor_tensor add/mult lower on Pool, so use broadcast scalars
            bc = lambda t: t[:, lt:lt + 1].to_broadcast((P, DV))
            mix = gat.tile([P, DV], F16, tag="gtmp", bufs=8, name="mix")
            nc.gpsimd.tensor_tensor(mix[:], vb_all[:, lt, :], bc(brec), op=ALU.mult)
            t1 = gat.tile([P, DV], F16, tag="gtmp", bufs=8, name="t1g")
            nc.gpsimd.tensor_tensor(t1[:], toks["ls"][:], bc(p1), op=ALU.mult)
            mix2 = gat.tile([P, DV], F16, tag="gtmp", bufs=8, name="mix2")
            nc.gpsimd.tensor_tensor(mix2[:], t1[:], mix[:], op=ALU.add)
            t2 = gat.tile([P, DV], F16, tag="gtmp", bufs=8, name="t2g")
            nc.gpsimd.tensor_tensor(t2[:], toks["ll"][:], bc(p2), op=ALU.mult)
            mix3 = gat.tile([P, DV], F16, tag="gtmp", bufs=8, name="mix3")
            nc.gpsimd.tensor_tensor(mix3[:], t2[:], mix2[:], op=ALU.add)
            tmp = gat.tile([P, DV], F16, tag="gtmp", bufs=8, name="tmpg")
            nc.gpsimd.tensor_tensor(tmp[:], mix3[:], bc(wg1m), op=ALU.mult)
            t3 = gat.tile([P, DV], F16, tag="gtmp", bufs=8, name="t3g")
            nc.gpsimd.tensor_tensor(t3[:], o_all[:, lt, :], bc(wg_c), op=ALU.mult)
            om = gat.tile([P, DV], F16, tag="gtmp", bufs=8, name="om")
            nc.gpsimd.tensor_tensor(om[:], t3[:], tmp[:], op=ALU.add)
            scr = gat.tile([P, DV], F16, tag="scr_g", bufs=2, name="scr_g")
            ssq = gat.tile([P, 1], F32, tag="ssq_g", bufs=2, name="ssq_g")
            nc.vector.scalar_tensor_tensor(scr[:], om[:], 1.0, om[:],
                                           op0=ALU.mult, op1=ALU.mult, accum_out=ssq[:])
            srt = gat.tile([P, 1], F32, tag="srt_g", bufs=2, name="srt_g")
            nc.scalar.activation(srt[:], ssq[:], AF.Sqrt, bias=eps_rms[:], scale=1.0 / DV)
            rin = gat.tile([P, 1], F32, tag="rin_g", bufs=2, name="rin_g")
            nc.vector.reciprocal(rin[:], srt[:])
            # output projection: transpose unnormalized om; fold the RMS scale
            # into the PSUM eviction (per-partition scale on out rows = tokens)
            onT = gat.tile([P, 2, CH], F16, tag="onT", bufs=2, name="onT")
            tp = ps_t.tile([P, 2, CH], F16, tag="pst", name="tp_on")
            for pt in range(2):
                nc.tensor.transpose(tp[:, pt, :], om[:, pt * P:(pt + 1) * P], ident[:])
            nc.scalar.copy(onT[:], tp[:])
            out_sb = gat.tile([P, D], F16, tag="out_sb", bufs=2, name="out_sb")
            for nt2 in range(2):
                ops = ps_big.tile([P, NTILE], F32, tag="psb", name="ops_o")
                for kt in range(2):
                    nc.tensor.matmul(ops[:], onT[:, kt, :],
                                     wo[:, kt, nt2 * NTILE:(nt2 + 1) * NTILE],
                                     start=(kt == 0), stop=(kt == 1))
                nc.scalar.activation(out_sb[:, nt2 * NTILE:(nt2 + 1) * NTILE], ops[:],
                                     AF.Identity, scale=rin[:])
            nc.sync.dma_start(out_d[lt * CH:(lt + 1) * CH, :], out_sb[:])

        # ---- steady-state emission: stages offset by iteration ----
        for it in range(NCH + 7):
            c4, c3, c2, c1 = it - 3, it - 2, it - 1, it
            gl = it - 7
            pend = list(fir_mm_thunks(it)) if it < NCH else []

            def drip(k):
                for _ in range(min(k, len(pend))):
                    pend.pop(0)()

            if 0 <= c4 < NCH:
                S4(c4)
            drip(2)
            if 0 <= c2 < NCH:
                S2(c2)
            if 0 <= c3 < NCH:
                S3(c3, drip)
            drip(len(pend))
            if it < NCH:
                fir_finish(it)
            if 0 <= c1 < NCH:
                S1(c1)
            if 0 <= gl < NCH:
                gate_out(gl, fch_of[gl // CPN])

    nc.compile()
    return nc


# ---------------- host side ----------------

def _diag_stack(taps):
    """taps [C, K] -> [2, K, 128, 128] diag matrices."""
    C, K = taps.shape
    out = np.zeros((2, K, P, P), np.float32)
    for pt in range(2):
        for j in range(K):
            np.fill_diagonal(out[pt, j], taps[pt * P:(pt + 1) * P, j])
    return out


def make_core_inputs(inputs, b, h, L):
    f = lambda a: np.ascontiguousarray(np.asarray(a, np.float32))
    x = f(inputs['hidden_states'])[b]          # [L, D]
    temp = float(np.exp(np.asarray(inputs['log_temp'], np.float64)[h]))
    wsm = np.concatenate([
        f(inputs['Wb'])[:, h:h + 1],
        f(inputs['Wg'])[:, h:h + 1],
        f(inputs['Wl'])[:, 3 * h:3 * h + 3] / temp], axis=1)
    bias5 = np.array([0.0, float(np.asarray(inputs['bg'], np.float64)[h]),
                      *(np.asarray(inputs['bl'], np.float64)[3 * h:3 * h + 3] / temp)],
                     np.float32)
    ct = np.stack([
        f(inputs['conv_q'])[h * DK:(h + 1) * DK].reshape(2, P, CONV_K),
        f(inputs['conv_k'])[h * DK:(h + 1) * DK].reshape(2, P, CONV_K),
        f(inputs['conv_v'])[h * DV:(h + 1) * DV].reshape(2, P, CONV_K)])  # [3, 2, 128, 4]
    # residual FIR taps: fir = delta + r  ->  local = v + FIR_r(v); softmax sums to 1
    fs = f(inputs['fir_short'])[h].copy()   # [DV, 3]
    fs[:, -1] -= 1.0
    fl = f(inputs['fir_long'])[h].copy()    # [DV, 31]
    fl[:, -1] -= 1.0
    fd = _diag_stack(fl).astype(np.float16)
    st = fs.reshape(2, P, FIR_S)
    wo = f(inputs['rms_w'])[:, None] * f(inputs['Wo'])[h * DV:(h + 1) * DV]
    h16 = np.float16
    return dict(
        xT=np.ascontiguousarray(x.T).astype(h16),
        wq=np.ascontiguousarray(f(inputs['Wq'])[:, h * DK:(h + 1) * DK]).astype(h16),
        wk=np.ascontiguousarray(f(inputs['Wk'])[:, h * DK:(h + 1) * DK]).astype(h16),
        wv=np.ascontiguousarray(f(inputs['Wv'])[:, h * DV:(h + 1) * DV]).astype(h16),
        wsm=wsm.astype(h16), bias5=bias5,
        ctaps=ct, fdiag=fd, staps=st.astype(np.float32), wo=wo.astype(h16),
        ident=np.eye(P, dtype=h16),
        masklt=np.tril(np.ones((P, P), h16), -1),
        maskut=np.triu(np.ones((P, P), h16), 0),
    )


_NC_CACHE = {}


def _get_nc(L):
    if L not in _NC_CACHE:
        _NC_CACHE[L] = build(L)
    return _NC_CACHE[L]


def kernel(**inputs):
    x = np.asarray(inputs['hidden_states'])
    Bx, L, _ = x.shape
    nc = _get_nc(L)
    in_maps = [make_core_inputs(inputs, c // H, c % H, L) for c in range(8)]
    res = run_bass_kernel_spmd(nc, in_maps, core_ids=list(range(8)))
    out = np.zeros((Bx, L, D), np.float32)
    for c in range(8):
        out[c // H] += res.results[c]['out'].astype(np.float32)
    return out


# revision 32
# speedup vs baseline: 2.0508x; 1.0703x over previous
"""DeltaNet Bass kernel for Trainium2, 8-core SPMD — fp16 matmul pipeline.

Sharding: core = (b, h) for b in 0..1, h in 0..3  (b*4 + h).
Each core computes the full per-(batch,head) pipeline and its partial
output projection out_partial[L, D] in fp16; the host sums the 4
head-partials per batch.

Device pipeline per core (PSUM accumulate fp32, operands fp16):
  phase A (per 512-col tile, all 8 tiles first): q/k/v/small projections
          (PE, fp16), causal 4-tap conv (DVE STT chain, fp16 4x mode),
          SiLU evict (ACT) into persistent SBUF: rollkq (chan-major k|q
          interleaved per 128-token chunk) and vwin (guarded chan-major v).
  phase B prologue: beta + gate columns from rows scratch (one Sigmoid /
          Exp table load each).
  phase B (per 128-token chunk): PE transposes (fp16 PSUM), l2norm stats,
          UT-transform T^T via Neumann product, chunk-local prepass,
          serial scan (u = u0 - w S, o = q S + attn u, S += k^T u) with
          fp32 S master + fp16 S copy; FIR-long diag matmuls + FIR-short
          STT chains interleaved per tile to fill PE gaps.
  phase C (per chunk, pipelined one tile behind): FIR transposes,
          hierarchical gating (DVE fp16), RMSNorm, output projection.
"""
import numpy as np
import ml_dtypes
from contextlib import ExitStack

import concourse.bass as bass
import concourse.tile as tile
from concourse import bacc, mybir
from concourse.bass_utils import run_bass_kernel_spmd

F32 = mybir.dt.float32
F16 = mybir.dt.float16
AF = mybir.ActivationFunctionType
ALU = mybir.AluOpType

B, D, H, DK, DV = 2, 1024, 4, 256, 256
CONV_K, FIR_S, FIR_L = 4, 3, 31
CH = 128          # scan chunk (token tile)
NTILE = 512       # column tile for projections / FIR
P = 128
KT = D // P       # 8 contraction tiles over D
GUARD = CONV_K - 1
EPS_RMS = 1e-5
FGUARD = FIR_L    # guard cols ahead of token 0 in vwin


def build(L=4096):
    NT = L // NTILE
    NCH = L // CH
    CPN = NTILE // CH  # chunks per n-tile (4)

    nc = bacc.Bacc("TRN2", target_bir_lowering=False, debug=False, num_devices=8)

    xT_d = nc.dram_tensor("xT", [D, L], F16, kind="ExternalInput").ap()
    wq_d = nc.dram_tensor("wq", [D, DK], F16, kind="ExternalInput").ap()
    wk_d = nc.dram_tensor("wk", [D, DK], F16, kind="ExternalInput").ap()
    wv_d = nc.dram_tensor("wv", [D, DV], F16, kind="ExternalInput").ap()
    wsm_d = nc.dram_tensor("wsm", [D, 5], F16, kind="ExternalInput").ap()
    bias5_d = nc.dram_tensor("bias5", [5], F32, kind="ExternalInput").ap()
    # conv taps per (tensor, pt): [3, 2, 128, 4]
    ctaps_d = nc.dram_tensor("ctaps", [3, 2, P, CONV_K], F32, kind="ExternalInput").ap()
    # fir long-residual diagonal matrices: [pt=2, 31, 128, 128]
    fdiag_d = nc.dram_tensor("fdiag", [2, FIR_L, P, P], F16, kind="ExternalInput").ap()
    # fir short-residual taps: [2, 128, 3]
    staps_d = nc.dram_tensor("staps", [2, P, FIR_S], F32, kind="ExternalInput").ap()
    wo_d = nc.dram_tensor("wo", [DV, D], F16, kind="ExternalInput").ap()
    ident_d = nc.dram_tensor("ident", [P, P], F16, kind="ExternalInput").ap()
    masklt_d = nc.dram_tensor("masklt", [P, P], F16, kind="ExternalInput").ap()  # strict lower
    maskut_d = nc.dram_tensor("maskut", [P, P], F16, kind="ExternalInput").ap()  # upper incl diag
    out_d = nc.dram_tensor("out", [L, D], F16, kind="ExternalOutput").ap()

    with tile.TileContext(nc) as tc, ExitStack() as ctx:
        # ---------------- pools ----------------
        const = ctx.enter_context(tc.tile_pool(name="const", bufs=1))
        bigw = ctx.enter_context(tc.tile_pool(name="bigw", bufs=1))
        persist = ctx.enter_context(tc.tile_pool(name="persist", bufs=1))
        xtp = ctx.enter_context(tc.tile_pool(name="xtp", bufs=2))
        prep = ctx.enter_context(tc.tile_pool(name="prep", bufs=1))
        colp = ctx.enter_context(tc.tile_pool(name="colp", bufs=1))
        chk = ctx.enter_context(tc.tile_pool(name="chk", bufs=1))
        sp = ctx.enter_context(tc.tile_pool(name="sp", bufs=1))
        gat = ctx.enter_context(tc.tile_pool(name="gat", bufs=1))
        dram = ctx.enter_context(tc.tile_pool(name="dram", bufs=1, space="DRAM"))
        ps_big = ctx.enter_context(tc.tile_pool(name="ps_big", bufs=2, space="PSUM"))
        ps_med = ctx.enter_context(tc.tile_pool(name="ps_med", bufs=3, space="PSUM"))
        ps_t = ctx.enter_context(tc.tile_pool(name="ps_t", bufs=2, space="PSUM"))
        ps_fir = ctx.enter_context(tc.tile_pool(name="ps_fir", bufs=1, space="PSUM"))



        # ---------------- constants / weights ----------------
        wq = bigw.tile([P, KT, DK], F16, tag="wq")
        nc.sync.dma_start(wq[:], wq_d.rearrange("(kt p) m -> p kt m", p=P))
        wk = bigw.tile([P, KT, DK], F16, tag="wk")
        nc.sync.dma_start(wk[:], wk_d.rearrange("(kt p) m -> p kt m", p=P))
        wv = bigw.tile([P, KT, DV], F16, tag="wv")
        nc.sync.dma_start(wv[:], wv_d.rearrange("(kt p) m -> p kt m", p=P))
        wsm = const.tile([P, KT, 5], F16)
        nc.sync.dma_start(wsm[:], wsm_d.rearrange("(kt p) m -> p kt m", p=P))
        ident = const.tile([P, P], F16)
        nc.sync.dma_start(ident[:], ident_d)
        masklt = const.tile([P, P], F16)
        nc.sync.dma_start(masklt[:], masklt_d)
        maskut = const.tile([P, P], F16)
        nc.sync.dma_start(maskut[:], maskut_d)
        bias5 = const.tile([5, 1], F32)
        nc.sync.dma_start(bias5[:], bias5_d.rearrange("(m o) -> m o", o=1))
        ctaps = const.tile([P, 3, 2, CONV_K], F32, name="ctaps")
        nc.sync.dma_start(ctaps[:], ctaps_d.rearrange("t pt p j -> p t pt j"))
        staps = const.tile([P, 2, FIR_S], F32, name="staps")
        nc.sync.dma_start(staps[:], staps_d.rearrange("pt p j -> p pt j"))
        fdiag = bigw.tile([P, 2, FIR_L, P], F16, tag="fdiag", name="fdiag")
        nc.sync.dma_start(fdiag[:], fdiag_d.rearrange("pt j p q -> p pt j q"))
        wo = bigw.tile([P, 2, D], F16, tag="wo", name="wo")
        nc.sync.dma_start(wo[:], wo_d.rearrange("(kt p) m -> p kt m", p=P))
        zeros3 = const.tile([P, GUARD], F16)
        nc.vector.memset(zeros3[:], 0.0)
        eps_l2 = const.tile([P, 1], F32)
        nc.vector.memset(eps_l2[:], 1e-6)
        eps_rms = const.tile([P, 1], F32)
        nc.vector.memset(eps_rms[:], EPS_RMS)

        # ---------------- persistent SBUF state ----------------
        # gate columns: [P, chunk, (beta_raw, wg_raw, l0, l1, l2, pad*3)] token-major
        gcols = persist.tile([P, NCH, 8], F16, name="gcols")
        # chan-major post-silu k|q interleaved per chunk: [P, pt, chunk, (k|q), CH]
        rollkq = persist.tile([P, 2, NCH, 2, CH], F16, name="rollkq")
        # chan-major post-silu v with FGUARD leading guard cols
        vwin = persist.tile([P, 2, FGUARD + L], F16, name="vwin")
        nc.vector.memset(vwin[:, :, 0:FGUARD], 0.0)
        # per-chunk outputs of the scan + beta-scaled v (token-major)
        o_all = persist.tile([P, NCH, DV], F16, name="o_all")
        vb_all = persist.tile([P, NCH, DV], F16, name="vb_all")

        S16 = sp.tile([P, 2, DV], F16, name="S16")

        TENS = ("q", "k", "v")
        W_OF = {"q": wq, "k": wk, "v": wv}
        prev_pre = {}

        # ================= phase A: projections + conv + silu =================
        def phaseA(n):
            xt = xtp.tile([P, KT, NTILE], F16, tag="xt", name="xt")
            nc.sync.dma_start(xt[:], xT_d[:, n * NTILE:(n + 1) * NTILE]
                              .rearrange("(kt p) t -> p kt t", p=P))
            for t in TENS:
                for pt in range(2):
                    ps = ps_big.tile([P, NTILE], F32, tag="psb", name=f"ps_{t}{pt}")
                    for kt in range(KT):
                        nc.tensor.matmul(ps[:], W_OF[t][:, kt, pt * P:(pt + 1) * P],
                                         xt[:, kt, :], start=(kt == 0), stop=(kt == KT - 1))
                    key = f"pre{t}{pt}"
                    pre = prep.tile([P, GUARD + NTILE], F16, tag=key, bufs=2, name=key)
                    if n == 0:
                        nc.scalar.copy(pre[:, 0:GUARD], zeros3[:])
                    else:
                        nc.scalar.copy(pre[:, 0:GUARD], prev_pre[key][:, NTILE:NTILE + GUARD])
                    nc.scalar.copy(pre[:, GUARD:], ps[:])
                    prev_pre[key] = pre
                    # conv: 4-tap chain. q/k on DVE in TS+TT form (tensor_scalar
                    # and tensor_tensor hit the 2x 16-bit DVE mode; STT does
                    # not). v on the idle Pool engine (TT-only, broadcast taps).
                    ti = TENS.index(t)
                    acc = prep.tile([P, NTILE], F16, tag="cacc", bufs=2, name="cacc")
                    if t == "v" and pt == 1:
                        bct = lambda j: ctaps[:, ti, pt, j:j + 1].to_broadcast((P, NTILE))
                        nc.gpsimd.tensor_tensor(acc[:], pre[:, 0:NTILE], bct(0), op=ALU.mult)
                        for j in range(1, CONV_K):
                            prod = prep.tile([P, NTILE], F16, tag="cprod", bufs=2, name="cprod")
                            nc.gpsimd.tensor_tensor(prod[:], pre[:, j:j + NTILE], bct(j),
                                                    op=ALU.mult)
                            nc.gpsimd.tensor_tensor(acc[:], acc[:], prod[:], op=ALU.add)
                    else:
                        nc.vector.tensor_scalar_mul(acc[:], pre[:, 0:NTILE],
                                                    ctaps[:, ti, pt, 0:1])
                        for j in range(1, CONV_K):
                            prod = prep.tile([P, NTILE], F16, tag="cprod", bufs=2, name="cprod")
                            nc.vector.tensor_scalar_mul(prod[:], pre[:, j:j + NTILE],
                                                        ctaps[:, ti, pt, j:j + 1])
                            nc.vector.tensor_tensor(acc[:], acc[:], prod[:], op=ALU.add)
                    # silu -> persistent layout
                    if t == "v":
                        nc.scalar.activation(
                            vwin[:, pt, FGUARD + n * NTILE: FGUARD + (n + 1) * NTILE],
                            acc[:], AF.Silu)
                    else:
                        koff = 0 if t == "k" else 1
                        nc.scalar.activation(
                            rollkq[:, pt, n * CPN:(n + 1) * CPN, koff, :],
                            acc[:], AF.Silu)
            # small projections [5, NTILE]
            ps5 = ps_big.tile([P, NTILE], F32, tag="psb", name="ps5")
            for kt in range(KT):
                nc.tensor.matmul(ps5[:5, :], wsm[:, kt, :], xt[:, kt, :],
                                 start=(kt == 0), stop=(kt == KT - 1))
            rows_sb = colp.tile([5, NTILE], F16, tag="rows_sb", bufs=2, name="rows_sb")
            nc.scalar.activation(rows_sb[:], ps5[:5, :], AF.Identity, bias=bias5[:])
            # token-major gate columns via tiny PE transposes (no DRAM round trip)
            rtp = ps_t.tile([P, 2, CH], F16, tag="pst", name="rtp")
            for ci in range(CPN):
                nc.tensor.transpose(rtp[:, 0, ci * 8:ci * 8 + 5],
                                    rows_sb[:, ci * CH:(ci + 1) * CH], ident[:5, :5])
            nc.scalar.copy(gcols[:, n * CPN:(n + 1) * CPN, :], rtp[:, 0, 0:8 * CPN])

        for n in range(NT):
            phaseA(n)

        # ================= phase B prologue: beta + gate columns =================
        beta_all = colp.tile([P, NCH], F32, tag="beta_all", bufs=1, name="beta_all")
        nc.scalar.activation(beta_all[:], gcols[:, :, 0], AF.Sigmoid)
        wg_c = colp.tile([P, NCH], F32, tag="wg_c", bufs=1, name="wg_c")
        nc.scalar.activation(wg_c[:], gcols[:, :, 1], AF.Sigmoid)
        mx = colp.tile([P, NCH], F32, tag="mx", bufs=1, name="mx")
        nc.vector.tensor_tensor(mx[:], gcols[:, :, 2], gcols[:, :, 3], op=ALU.max)
        nc.vector.tensor_tensor(mx[:], mx[:], gcols[:, :, 4], op=ALU.max)
        e0 = colp.tile([P, NCH], F32, tag="e0", bufs=1, name="e0")
        e1 = colp.tile([P, NCH], F32, tag="e1", bufs=1, name="e1")
        e2 = colp.tile([P, NCH], F32, tag="e2", bufs=1, name="e2")
        for r, dst in ((2, e0), (3, e1), (4, e2)):
            nc.vector.tensor_tensor(dst[:], gcols[:, :, r], mx[:], op=ALU.subtract)
            nc.scalar.activation(dst[:], dst[:], AF.Exp)
        esum = colp.tile([P, NCH], F32, tag="esum", bufs=1, name="esum")
        nc.vector.tensor_tensor(esum[:], e0[:], e1[:], op=ALU.add)
        nc.vector.tensor_tensor(esum[:], esum[:], e2[:], op=ALU.add)
        erec = colp.tile([P, NCH], F32, tag="erec", bufs=1, name="erec")
        nc.vector.reciprocal(erec[:], esum[:])
        p1 = colp.tile([P, NCH], F32, tag="p1", bufs=1, name="p1")
        p2 = colp.tile([P, NCH], F32, tag="p2", bufs=1, name="p2")
        for src, dst in ((e1, p1), (e2, p2)):
            nc.vector.tensor_tensor(dst[:], src[:], erec[:], op=ALU.mult)
        brec = colp.tile([P, NCH], F32, tag="brec", bufs=1, name="brec")
        nc.vector.reciprocal(brec[:], beta_all[:])
        wg1m = colp.tile([P, NCH], F32, tag="wg1m", bufs=1, name="wg1m")
        nc.vector.tensor_scalar(wg1m[:], wg_c[:], -1.0, 1.0, op0=ALU.mult, op1=ALU.add)

        # ================= phase B: per-chunk scan =================
        def vtok_col(c):
            return vwin[:, :, FGUARD + c * CH: FGUARD + (c + 1) * CH]

        # ---- software-pipelined chunk stages ----
        # S1(c): token-major stats + G  (emitted 3 iterations ahead)
        # S2(c): scalings N1/Mt/Nt/attnT/khat/khatb  (2 ahead)
        # S3(c): Neumann ladder + P-chain + wTn/u0  (1 ahead)
        # S4(c): serial scan step  (current)
        # FIR-long matmuls are dripped between dependent S3 steps so the
        # in-order PE queue always has ready work.
        st1, st2, st3 = {}, {}, {}

        def S1(c):
            beta_c = beta_all[:, c:c + 1]
            res = {"beta": beta_c}
            tpv = ps_t.tile([P, 2, CH], F16, tag="pst", name="tp_v")
            for pt in range(2):
                nc.tensor.transpose(tpv[:, pt, :],
                                    vwin[:, pt, FGUARD + c * CH: FGUARD + (c + 1) * CH],
                                    ident[:])
            nc.vector.tensor_scalar_mul(vb_all[:, c, :], tpv[:], beta_c)
            res["kTsl"] = [rollkq[:, pt, c, 0, :] for pt in range(2)]
            res["qTsl"] = [rollkq[:, pt, c, 1, :] for pt in range(2)]
            res["kqTsl"] = [rollkq[:, pt, c, :, :] for pt in range(2)]
            toks = {}
            for t, koff in (("q", 1), ("k", 0)):
                tok = chk.tile([P, DV], F16, tag=f"{t}tok", bufs=3, name=f"{t}tok")
                tpt = ps_t.tile([P, 2, CH], F16, tag="pst", name=f"tp_{t}")
                for pt in range(2):
                    nc.tensor.transpose(tpt[:, pt, :], rollkq[:, pt, c, koff, :], ident[:])
                nc.scalar.copy(tok[:], tpt[:])
                toks[t] = tok
            res["ktok"] = toks["k"]
            # [Graw | Braw] = kraw @ [kraw | qraw]^T ; evict fp16 to free the bank
            gps = ps_med.tile([P, DV], F32, tag="psm", name="gps")
            for pt in range(2):
                nc.tensor.matmul(gps[:], res["kTsl"][pt], res["kqTsl"][pt],
                                 start=(pt == 0), stop=(pt == 1))
            g16 = chk.tile([P, DV], F16, tag="g16", bufs=3, name="g16")
            nc.scalar.copy(g16[:], gps[:])
            res["g16"] = g16
            for t in ("q", "k"):
                scr = chk.tile([P, DV], F16, tag="sq_scr", bufs=2, name="sq_scr")
                ssq = chk.tile([P, 1], F32, tag=f"ssq{t}", bufs=2, name=f"ssq{t}")
                nc.vector.scalar_tensor_tensor(scr[:], toks[t][:], 1.0, toks[t][:],
                                               op0=ALU.mult, op1=ALU.mult, accum_out=ssq[:])
                sroot = chk.tile([P, 1], F32, tag=f"sroot{t}", bufs=2, name=f"sroot{t}")
                nc.scalar.activation(sroot[:], ssq[:], AF.Sqrt, bias=eps_l2[:])
                rinv = chk.tile([P, 1], F32, tag=f"rinv{t}", bufs=5, name=f"rinv{t}")
                nc.vector.reciprocal(rinv[:], sroot[:])
                res["rinv" + t] = rinv
            st1[c] = res

        def S2(c):
            res = st1.pop(c)
            beta_c, rinvk = res["beta"], res["rinvk"]
            khat = chk.tile([P, DV], F16, tag="khat", bufs=4, name="khat")
            nc.vector.tensor_scalar_mul(khat[:], res["ktok"][:], rinvk[:])
            res["khat"] = khat
            khatb = chk.tile([P, DV], F16, tag="khatb", bufs=3, name="khatb")
            nc.vector.tensor_scalar_mul(khatb[:], khat[:], beta_c)
            res["khatb"] = khatb
            br = chk.tile([P, 1], F32, tag="br", bufs=2, name="br")
            nc.vector.tensor_tensor(br[:], beta_c, rinvk[:], op=ALU.mult)
            N1 = chk.tile([P, P], F16, tag="N1", bufs=2, name="N1")
            nc.vector.scalar_tensor_tensor(N1[:], res["g16"][:, :P], br[:], masklt[:],
                                           op0=ALU.mult, op1=ALU.mult)
            mps = ps_t.tile([P, 2, CH], F16, tag="pst", name="mps")
            nc.tensor.transpose(mps[:, 0, :], N1[:], ident[:])
            Mt = chk.tile([P, P], F16, tag="Mt", bufs=3, name="Mt")
            nc.vector.tensor_scalar_mul(Mt[:], mps[:, 0, :], rinvk[:])
            nc.tensor.transpose(mps[:, 1, :], Mt[:], ident[:])
            Nt = chk.tile([P, P], F16, tag="Nt", bufs=3, name="Nt")
            nc.scalar.copy(Nt[:], mps[:, 1, :])
            res["Mt"], res["Nt"] = Mt, Nt
            attnT = chk.tile([P, P], F16, tag="attnT", bufs=4, name="attnT")
            nc.vector.scalar_tensor_tensor(attnT[:], res["g16"][:, P:], rinvk[:], maskut[:],
                                           op0=ALU.mult, op1=ALU.mult)
            res["attnT"] = attnT
            st2[c] = res

        def mm_small(lhsT, rhs, name, evict_eng):
            ps = ps_med.tile([P, DV], F32, tag="psm", name=f"ps_{name}")
            nc.tensor.matmul(ps[:, :P], lhsT, rhs, start=True, stop=True)
            sb = chk.tile([P, P], F16, tag=name, bufs=2, name=name)
            if evict_eng == "v":
                nc.vector.tensor_copy(sb[:], ps[:, :P])
            else:
                nc.scalar.copy(sb[:], ps[:, :P])
            return sb

        def S3(c, drip):
            res = st2.pop(c)
            Mt, Nt = res["Mt"], res["Nt"]
            # Neumann truncated at A^15: T^T = (I-Mt)(I+Mt^2)(I+Mt^4)(I+Mt^8)
            P1 = chk.tile([P, P], F16, tag="P1", bufs=2, name="P1")
            nc.vector.tensor_tensor(P1[:], ident[:], Mt[:], op=ALU.subtract)
            N2 = mm_small(Mt[:], Nt[:], "N2", "v")
            M2 = mm_small(Nt[:], Mt[:], "M2", "s")
            drip(3)
            N4 = mm_small(M2[:], N2[:], "N4", "s")
            M4 = mm_small(N2[:], M2[:], "M4", "s")
            drip(3)
            N8 = mm_small(M4[:], N4[:], "N8", "v")
            drip(2)
            Pc = P1
            for Npow, nm in ((N2, "P2"), (N4, "P3"), (N8, "TTt")):
                pps = ps_med.tile([P, DV], F32, tag="psm", name=f"pps_{nm}")
                nc.tensor.matmul(pps[:, :P], Npow[:], Pc[:], start=True, stop=True)
                nxt = chk.tile([P, P], F16, tag=nm, bufs=2, name=nm)
                nc.vector.tensor_tensor(nxt[:], Pc[:], pps[:, :P], op=ALU.add)
                Pc = nxt
                drip(2)
            TTt = Pc
            negTT = chk.tile([P, P], F16, tag="negTT", bufs=2, name="negTT")
            nc.vector.tensor_scalar_mul(negTT[:], TTt[:], -1.0)
            drip(2)
            wTn = chk.tile([P, 2, CH], F16, tag="wTn", bufs=3, name="wTn")
            for kt in range(2):
                wps = ps_med.tile([P, DV], F32, tag="psm", name="wps")
                nc.tensor.matmul(wps[:, :P], res["khatb"][:, kt * P:(kt + 1) * P], negTT[:],
                                 start=True, stop=True)
                nc.scalar.copy(wTn[:, kt, :], wps[:, :P])
            res["wTn"] = wTn
            ups = ps_med.tile([P, DV], F32, tag="psm", name="ups")
            nc.tensor.matmul(ups[:], TTt[:], vb_all[:, c, :], start=True, stop=True)
            u0 = chk.tile([P, DV], F16, tag="u0", bufs=3, name="u0")
            nc.scalar.copy(u0[:], ups[:])
            res["u0"] = u0
            st3[c] = res

        def S4(c):
            pr = st3.pop(c)
            if c == 0:
                u16 = pr["u0"]
            else:
                ups = ps_med.tile([P, DV], F32, tag="psm", name="ups_s")
                for kt in range(2):
                    nc.tensor.matmul(ups[:], pr["wTn"][:, kt, :], S16[:, kt, :],
                                     start=(kt == 0), stop=(kt == 1))
                u16 = chk.tile([P, DV], F16, tag="u16", bufs=2, name="u16")
                nc.vector.tensor_tensor(u16[:], ups[:], pr["u0"][:], op=ALU.add)
            ops = ps_med.tile([P, DV], F32, tag="psm", name="ops")
            if c == 0:
                nc.tensor.matmul(ops[:], pr["attnT"][:], u16[:], start=True, stop=True)
            else:
                for kt in range(2):
                    nc.tensor.matmul(ops[:], pr["qTsl"][kt], S16[:, kt, :],
                                     start=(kt == 0), stop=False)
                nc.tensor.matmul(ops[:], pr["attnT"][:], u16[:], start=False, stop=True)
            nc.vector.tensor_scalar_mul(o_all[:, c, :], ops[:], pr["rinvq"][:])
            # S += k^T u  (both kt halves in one PSUM bank, one fp16 add)
            dps = ps_big.tile([P, 2, DV], F32, tag="psb", name="dps")
            for kt in range(2):
                nc.tensor.matmul(dps[:, kt, :], pr["khat"][:, kt * P:(kt + 1) * P], u16[:],
                                 start=True, stop=True)
            if c == 0:
                nc.vector.tensor_copy(S16[:], dps[:])
            else:
                nc.vector.tensor_tensor(S16[:], S16[:], dps[:], op=ALU.add)

        # ---- FIR long: per-iteration batches of PE matmuls, dripped via S3 ----
        fir_state = {}
        fch_of = {}   # n -> fch dict

        def fir_mm_thunks(k):
            """Iteration k (0..NCH-1): list of closures, one FIR matmul each."""
            n, b = k // CPN, k % CPN
            pt, half = b // 2, b % 2
            st = fir_state.setdefault(n, {})
            thunks = []
            if half == 0:
                ps = ps_fir.tile([P, NTILE], F32, tag="fir", name="ps_ll")
                st[pt] = ps
                taps = range(0, 16)
            else:
                ps = st[pt]
                taps = range(16, FIR_L)
            for j in taps:
                def mk(j=j, ps=ps, pt=pt, n=n):
                    nc.tensor.matmul(ps[:], fdiag[:, pt, j, :],
                                     vwin[:, pt, n * NTILE + 1 + j: n * NTILE + 1 + j + NTILE],
                                     start=(j == 0), stop=(j == FIR_L - 1))
                thunks.append(mk)
            return thunks

        def fir_finish(k):
            """Evicts + FIR-short chain for iteration k's batch."""
            n, b = k // CPN, k % CPN
            pt, half = b // 2, b % 2
            fch = fch_of.setdefault(n, {})
            if half == 1:
                ll = gat.tile([P, NTILE], F16, tag="llch", bufs=6, name="llch")
                nc.scalar.copy(ll[:], fir_state[n][pt][:])
                fch[("ll", pt)] = ll
            else:
                ls = gat.tile([P, NTILE], F16, tag="lsch", bufs=6, name="lsch")
                bs = FGUARD - FIR_S + 1 + n * NTILE
                nc.vector.tensor_scalar_mul(ls[:], vwin[:, pt, bs:bs + NTILE],
                                            staps[:, pt, 0:1])
                for j in range(1, FIR_S):
                    sprod = gat.tile([P, NTILE], F16, tag="sprod", bufs=2, name="sprod")
                    nc.vector.tensor_scalar_mul(sprod[:], vwin[:, pt, bs + j:bs + j + NTILE],
                                                staps[:, pt, j:j + 1])
                    nc.vector.tensor_tensor(ls[:], ls[:], sprod[:], op=ALU.add)
                fch[("ls", pt)] = ls

        # ================= phase C: gating + output projection =================
        def gate_out(lt, fch):
            off = (lt % CPN) * CH
            cs = lambda t: t[:, lt:lt + 1]
            toks = {}
            for f in ("ls", "ll"):
                tokt = gat.tile([P, DV], F16, tag=f"{f}tok", bufs=2, name=f"{f}tok")
                tp = ps_t.tile([P, 2, CH], F16, tag="pst", name=f"tp_{f}")
                for pt in range(2):
                    nc.tensor.transpose(tp[:, pt, :], fch[(f, pt)][:, off:off + CH], ident[:])
                nc.scalar.copy(tokt[:], tp[:])
                toks[f] = tokt
            # gating chain on the (otherwise idle) GPSIMD engine: only plain
            # tensor_tensor add/mult lower on Pool, so use broadcast scalars
            bc = lambda t: t[:, lt:lt + 1].to_broadcast((P, DV))
            mix = gat.tile([P, DV], F16, tag="gtmp", bufs=8, name="mix")
            nc.gpsimd.tensor_tensor(mix[:], vb_all[:, lt, :], bc(brec), op=ALU.mult)
            t1 = gat.tile([P, DV], F16, tag="gtmp", bufs=8, name="t1g")
            nc.gpsimd.tensor_tensor(t1[:], toks["ls"][:], bc(p1), op=ALU.mult)
            mix2 = gat.tile([P, DV], F16, tag="gtmp", bufs=8, name="mix2")
            nc.gpsimd.tensor_tensor(mix2[:], t1[:], mix[:], op=ALU.add)
            t2 = gat.tile([P, DV], F16, tag="gtmp", bufs=8, name="t2g")
            nc.gpsimd.tensor_tensor(t2[:], toks["ll"][:], bc(p2), op=ALU.mult)
            mix3 = gat.tile([P, DV], F16, tag="gtmp", bufs=8, name="mix3")
            nc.gpsimd.tensor_tensor(mix3[:], t2[:], mix2[:], op=ALU.add)
            tmp = gat.tile([P, DV], F16, tag="gtmp", bufs=8, name="tmpg")
            nc.gpsimd.tensor_tensor(tmp[:], mix3[:], bc(wg1m), op=ALU.mult)
            t3 = gat.tile([P, DV], F16, tag="gtmp", bufs=8, name="t3g")
            nc.gpsimd.tensor_tensor(t3[:], o_all[:, lt, :], bc(wg_c), op=ALU.mult)
            om = gat.tile([P, DV], F16, tag="gtmp", bufs=8, name="om")
            nc.gpsimd.tensor_tensor(om[:], t3[:], tmp[:], op=ALU.add)
            scr = gat.tile([P, DV], F16, tag="scr_g", bufs=2, name="scr_g")
            ssq = gat.tile([P, 1], F32, tag="ssq_g", bufs=2, name="ssq_g")
            nc.vector.scalar_tensor_tensor(scr[:], om[:], 1.0, om[:],
                                           op0=ALU.mult, op1=ALU.mult, accum_out=ssq[:])
            srt = gat.tile([P, 1], F32, tag="srt_g", bufs=2, name="srt_g")
            nc.scalar.activation(srt[:], ssq[:], AF.Sqrt, bias=eps_rms[:], scale=1.0 / DV)
            rin = gat.tile([P, 1], F32, tag="rin_g", bufs=2, name="rin_g")
            nc.vector.reciprocal(rin[:], srt[:])
            # output projection: transpose unnormalized om; fold the RMS scale
            # into the PSUM eviction (per-partition scale on out rows = tokens)
            onT = gat.tile([P, 2, CH], F16, tag="onT", bufs=2, name="onT")
            tp = ps_t.tile([P, 2, CH], F16, tag="pst", name="tp_on")
            for pt in range(2):
                nc.tensor.transpose(tp[:, pt, :], om[:, pt * P:(pt + 1) * P], ident[:])
            nc.scalar.copy(onT[:], tp[:])
            out_sb = gat.tile([P, D], F16, tag="out_sb", bufs=2, name="out_sb")
            for nt2 in range(2):
                ops = ps_big.tile([P, NTILE], F32, tag="psb", name="ops_o")
                for kt in range(2):
                    nc.tensor.matmul(ops[:], onT[:, kt, :],
                                     wo[:, kt, nt2 * NTILE:(nt2 + 1) * NTILE],
                                     start=(kt == 0), stop=(kt == 1))
                nc.scalar.activation(out_sb[:, nt2 * NTILE:(nt2 + 1) * NTILE], ops[:],
                                     AF.Identity, scale=rin[:])
            nc.sync.dma_start(out_d[lt * CH:(lt + 1) * CH, :], out_sb[:])

        # ---- steady-state emission: stages offset by iteration ----
        for it in range(NCH + 5):
            c4, c3, c2, c1 = it - 3, it - 2, it - 1, it
            gl = it - 5
            pend = list(fir_mm_thunks(it)) if it < NCH else []

            def drip(k):
                for _ in range(min(k, len(pend))):
                    pend.pop(0)()

            if 0 <= c4 < NCH:
                S4(c4)
            drip(2)
            if 0 <= c2 < NCH:
                S2(c2)
            if 0 <= c3 < NCH:
                S3(c3, drip)
            drip(len(pend))
            if it < NCH:
                fir_finish(it)
            if 0 <= c1 < NCH:
                S1(c1)
            if 0 <= gl < NCH:
                gate_out(gl, fch_of[gl // CPN])

    nc.compile()
    return nc


# ---------------- host side ----------------

def _diag_stack(taps):
    """taps [C, K] -> [2, K, 128, 128] diag matrices."""
    C, K = taps.shape
    out = np.zeros((2, K, P, P), np.float32)
    for pt in range(2):
        for j in range(K):
            np.fill_diagonal(out[pt, j], taps[pt * P:(pt + 1) * P, j])
    return out


def make_core_inputs(inputs, b, h, L):
    f = lambda a: np.ascontiguousarray(np.asarray(a, np.float32))
    x = f(inputs['hidden_states'])[b]          # [L, D]
    temp = float(np.exp(np.asarray(inputs['log_temp'], np.float64)[h]))
    wsm = np.concatenate([
        f(inputs['Wb'])[:, h:h + 1],
        f(inputs['Wg'])[:, h:h + 1],
        f(inputs['Wl'])[:, 3 * h:3 * h + 3] / temp], axis=1)
    bias5 = np.array([0.0, float(np.asarray(inputs['bg'], np.float64)[h]),
                      *(np.asarray(inputs['bl'], np.float64)[3 * h:3 * h + 3] / temp)],
                     np.float32)
    ct = np.stack([
        f(inputs['conv_q'])[h * DK:(h + 1) * DK].reshape(2, P, CONV_K),
        f(inputs['conv_k'])[h * DK:(h + 1) * DK].reshape(2, P, CONV_K),
        f(inputs['conv_v'])[h * DV:(h + 1) * DV].reshape(2, P, CONV_K)])  # [3, 2, 128, 4]
    # residual FIR taps: fir = delta + r  ->  local = v + FIR_r(v); softmax sums to 1
    fs = f(inputs['fir_short'])[h].copy()   # [DV, 3]
    fs[:, -1] -= 1.0
    fl = f(inputs['fir_long'])[h].copy()    # [DV, 31]
    fl[:, -1] -= 1.0
    fd = _diag_stack(fl).astype(np.float16)
    st = fs.reshape(2, P, FIR_S)
    wo = f(inputs['rms_w'])[:, None] * f(inputs['Wo'])[h * DV:(h + 1) * DV]
    h16 = np.float16
    return dict(
        xT=np.ascontiguousarray(x.T).astype(h16),
        wq=np.ascontiguousarray(f(inputs['Wq'])[:, h * DK:(h + 1) * DK]).astype(h16),
        wk=np.ascontiguousarray(f(inputs['Wk'])[:, h * DK:(h + 1) * DK]).astype(h16),
        wv=np.ascontiguousarray(f(inputs['Wv'])[:, h * DV:(h + 1) * DV]).astype(h16),
        wsm=wsm.astype(h16), bias5=bias5,
        ctaps=ct, fdiag=fd, staps=st.astype(np.float32), wo=wo.astype(h16),
        ident=np.eye(P, dtype=h16),
        masklt=np.tril(np.ones((P, P), h16), -1),
        maskut=np.triu(np.ones((P, P), h16), 0),
    )


_NC_CACHE = {}


def _get_nc(L):
    if L not in _NC_CACHE:
        _NC_CACHE[L] = build(L)
    return _NC_CACHE[L]


def kernel(**inputs):
    x = np.asarray(inputs['hidden_states'])
    Bx, L, _ = x.shape
    nc = _get_nc(L)
    in_maps = [make_core_inputs(inputs, c // H, c % H, L) for c in range(8)]
    res = run_bass_kernel_spmd(nc, in_maps, core_ids=list(range(8)))
    out = np.zeros((Bx, L, D), np.float32)
    for c in range(8):
        out[c // H] += res.results[c]['out'].astype(np.float32)
    return out
